# revision 26
# baseline (speedup 1.0000x reference)
"""Multi-head attention kernel for Trainium2, SPMD over 8 NeuronCores.

Problem: B=2, N=4096, C=512, H=8 heads, DH=64. fp32 I/O.
Sharding: core c -> batch b=c//4, heads {2*(c%4), 2*(c%4)+1}.
Each core computes its 2 heads' attention + a partial output projection
(transposed layout [C, N], bf16); the host sums the 4 partials per batch
and transposes back.

v2 design (ACT was the bottleneck at 267us busy / 327us total):
- S^T matmuls run in fp8e4 DoubleRow (0.5 cyc/row): q,k are projected,
  scaled by 16 and quantized to fp8 in a [32, 2, N] layout (contraction
  DH=64 split as Ki=32 partitions x Ko=2), halving the dominant PE term.
- ~31% of the exp tiles are offloaded from ACT: softmax weights for
  those (kv, head) tiles use the 2nd-order Taylor form
  (1+S)^2 + 1 = 2*exp(S) + O(S^3)  (S has std ~0.07, |S|<0.65)
  computed as one DVE tensor_scalar (t = 1 + S, reading PSUM) plus one
  GPSIMD tensor_tensor (t*t). ACT tiles compute 2*exp(S) via bias=ln2,
  and the "+1" constant is folded in algebraically: a per-head
  sum-of-v over the Taylor kv tiles (sv) is accumulated into the o/den
  accumulators with K=1 broadcast matmuls, so normalization divides the
  consistent 2x-scaled numerator/denominator.
- next-iteration pT producers are emitted before the boundary norm work
  so ACT/DVE/GPSIMD never wait on the norm chain; norm uses batched
  reciprocals and ping-pong transpose slots carved out of accC's free
  PSUM space.
- startup: PE-warmup transpose chain (p-state ramp), x chunk0 + the
  q/k halves of w are DMA'd first across 3 queues.
"""

import math
from collections import deque

import numpy as np
import ml_dtypes

import concourse.tile as tile
from concourse import bacc, mybir
from concourse.bass_utils import run_bass_kernel_spmd
from concourse.masks import make_identity

BF16 = ml_dtypes.bfloat16
E4M3 = ml_dtypes.float8_e4m3fn

B, N, C, H = 2, 4096, 512, 8
DH = C // H          # 64
NCORES = 8
SCALE = C ** -0.5    # reference scales by hidden_dim, not head_dim
PRE = 16.0           # q/k prescale into fp8e4 range
EXPSCALE = SCALE / (PRE * PRE)
LN2 = math.log(2.0)

QS = 1024            # query superblock
NQS = N // QS        # 4
NKV = N // 128       # 32 kv tiles
NQT = QS // 128      # 8 query tiles per superblock
CH = 512             # token chunk for projections
NCH = N // CH        # 8

FP32 = mybir.dt.float32
BF16_DT = mybir.dt.bfloat16
FP8 = mybir.dt.float8e4

# Taylor (DVE+GPSIMD) kv tiles per head; same sets for every superblock.
TK = (frozenset(range(0, 30, 3)), frozenset(range(1, 31, 3)))  # 10 + 10

PSV0 = 136     # fp32 col in accC where the sv accumulator lives [136:266)
TRB0 = 544     # bf16 col in accC for transpose ping-pong slots (2x128)
WARMUP = 40    # PE p-state warmup transposes

ADD = mybir.AluOpType.add
MULT = mybir.AluOpType.mult


def _emit(tc):
    nc = tc.nc
    xT = nc.dram_tensor("xT", [C, N], BF16_DT, kind="ExternalInput").ap()
    wqkv = nc.dram_tensor("wqkv", [C, 6 * DH], BF16_DT, kind="ExternalInput").ap()
    bqkv = nc.dram_tensor("bqkv", [3, 128], FP32, kind="ExternalInput").ap()
    wout = nc.dram_tensor("wout", [DH, 2 * C], BF16_DT, kind="ExternalInput").ap()
    bout = nc.dram_tensor("bout", [4, 128], FP32, kind="ExternalInput").ap()
    poutT = nc.dram_tensor("poutT", [C, N], BF16_DT, kind="ExternalOutput").ap()

    with (
        tc.tile_pool(name="singles", bufs=1) as singles,
        tc.tile_pool(name="psum_big", bufs=1, space="PSUM") as pbig,
        tc.tile_pool(name="psum_sT", bufs=2, space="PSUM") as psT,
        tc.tile_pool(name="psum_acc", bufs=1, space="PSUM") as pacc,
        # separate pT pools per producer engine: a shared pool would create
        # WAW buffer deps chaining ACT behind the GPSIMD tt stream
        tc.tile_pool(name="pT_act", bufs=5) as ppT_act,
        tc.tile_pool(name="pT_tay", bufs=4) as ppT_tay,
        tc.tile_pool(name="tay_pool", bufs=6) as ptay,
        tc.tile_pool(name="q8_pool", bufs=3) as pq8,
        tc.tile_pool(name="norm_pool", bufs=4) as pnorm,
        tc.tile_pool(name="o_pool", bufs=18) as po,
        tc.tile_pool(name="stage_out", bufs=4) as so,
    ):
        # --- resident SBUF tensors ---
        xT_sb = singles.tile([128, 4, N], BF16_DT)
        w_sb = singles.tile([128, 4, 6 * DH], BF16_DT)
        bq_sb = singles.tile([128, 3], FP32)
        wo_sb = singles.tile([128, 2 * C], BF16_DT)
        bo_sb = singles.tile([128, 4], FP32)
        ident = singles.tile([128, 128], BF16_DT)
        ones_col = singles.tile([128, 1], BF16_DT)
        ones_row = singles.tile([1, 128], FP32)
        # q/k fp8 [32,2]-split: parts 0-31 h0, 32-63 h1; free (j, h... )
        # element (p, j, n) = PRE * q_{d = 32j + p}(n), head = p//32
        q8_sb = singles.tile([64, 2, N], FP8)
        k8_sb = singles.tile([64, 2, N], FP8)
        vT_sb = singles.tile([128, N], BF16_DT)
        v_sb = singles.tile([128, NKV, 130], BF16_DT)
        oT_sb = singles.tile([64, 2 * N], BF16_DT)
        sv_sb = singles.tile([1, 130], FP32)
        ln2_sb = singles.tile([128, 1], FP32)
        warm = singles.tile([128, 1], FP32)

        make_identity(nc, ident)
        nc.vector.memset(ones_col, 1.0)
        nc.vector.memset(ones_row, 1.0)
        nc.vector.memset(ln2_sb, float(LN2))
        nc.vector.memset(v_sb[:, :, 64:65], 1.0)
        nc.vector.memset(v_sb[:, :, 129:130], 1.0)
        nc.vector.memset(warm, 0.0)
        # load the Exp table set during setup
        nc.scalar.activation(out=warm, in_=warm,
                             func=mybir.ActivationFunctionType.Exp)
        # PE p-state warmup: keep PE continuously busy through the DMA wait
        for _ in range(WARMUP):
            wt = pbig.tile([128, 128], BF16_DT, tag="big", name="warm")
            nc.tensor.transpose(wt, ident, ident)

        # --- DMA schedule: x ch0 + w(qk) first, on 3 HWDGE queues ---
        QQ = (nc.sync, nc.scalar, nc.gpsimd)

        def x_piece(ch, kt):
            # sync/scalar HWDGE only: gpsimd's DMA issue occupies the Pool
            # engine, which the Taylor tt stream needs
            QQ[(4 * ch + kt) % 2].dma_start(
                out=xT_sb[:, kt, CH * ch:CH * (ch + 1)],
                in_=xT[128 * kt:128 * (kt + 1), CH * ch:CH * (ch + 1)])

        for kt in range(4):
            x_piece(0, kt)
        for kt in range(4):
            (nc.sync if kt % 2 else nc.scalar).dma_start(
                out=w_sb[:, kt, 0:256], in_=wqkv[128 * kt:128 * (kt + 1), 0:256])
        for j in range(3):
            nc.scalar.dma_start(out=bq_sb[:, j:j + 1], in_=bqkv[j, :])
        for kt in range(4):
            x_piece(1, kt)
        for kt in range(4):
            (nc.sync if kt % 2 else nc.scalar).dma_start(
                out=w_sb[:, kt, 256:384],
                in_=wqkv[128 * kt:128 * (kt + 1), 256:384])
        nc.sync.dma_start(out=wo_sb[0:DH, :], in_=wout[:, :])
        for j in range(4):
            nc.scalar.dma_start(out=bo_sb[:, j:j + 1], in_=bout[j, :])
        for ch in range(2, NCH):
            for kt in range(4):
                x_piece(ch, kt)

        # ---------- emission helpers ----------

        def proj_qk(dst8, wcol0, bcol, ch, pool=None):
            """Project one 512-token chunk of q or k into fp8 [32,2] layout."""
            sl = slice(CH * ch, CH * (ch + 1))
            if pool is None:
                ps = pbig.tile([128, CH], FP32, tag="big", name="ps")
            else:
                ps = pool.tile([128, CH], FP32, tag="sT", name="ps")
            for kt in range(4):
                nc.tensor.matmul(
                    ps, lhsT=w_sb[:, kt, wcol0:wcol0 + 128],
                    rhs=xT_sb[:, kt, sl], start=(kt == 0), stop=(kt == 3))
            # rows 0-63 = j0 (both heads): convert in place
            nc.vector.tensor_scalar(
                out=dst8[0:64, 0, sl], in0=ps[0:64, :],
                scalar1=bq_sb[0:64, bcol:bcol + 1], scalar2=float(PRE),
                op0=ADD, op1=MULT)
            # rows 64-127 = j1: convert, then partition-shift via DMA
            qt8 = pq8.tile([128, CH], FP8, tag="q8")
            nc.vector.tensor_scalar(
                out=qt8[64:128, :], in0=ps[64:128, :],
                scalar1=bq_sb[64:128, bcol:bcol + 1], scalar2=float(PRE),
                op0=ADD, op1=MULT)
            nc.scalar.dma_start(out=dst8[0:64, 1, sl], in_=qt8[64:128, :])

        def proj_v(ch, pool=None):
            sl = slice(CH * ch, CH * (ch + 1))
            if pool is None:
                ps = pbig.tile([128, CH], FP32, tag="big", name="ps")
            else:
                ps = pool.tile([128, CH], FP32, tag="sT", name="ps")
            for kt in range(4):
                nc.tensor.matmul(
                    ps, lhsT=w_sb[:, kt, 256:384],
                    rhs=xT_sb[:, kt, sl], start=(kt == 0), stop=(kt == 3))
            nc.vector.tensor_scalar_add(
                out=vT_sb[:, sl], in0=ps, scalar1=bq_sb[:, 2:3])

        def vtr(kv, pool=None):
            """Transpose v^T tile kv into v_sb [tok, d] layout."""
            if pool is None:
                trp = pbig.tile([128, 128], BF16_DT, tag="big", name="trp")
            else:
                trp = pool.tile([128, 128], BF16_DT, tag="sT", name="trp")
            nc.tensor.transpose(trp, vT_sb[:, 128 * kv:128 * (kv + 1)], ident)
            src = trp.rearrange("p (j c) -> p j c", j=2)
            dst = v_sb[:, kv, 0:130].rearrange("p (j c) -> p j c", j=2)
            nc.vector.tensor_copy(out=dst[:, :, 0:64], in_=src)

        def s_mm(qs, kv, h):
            """S^T = k_tile^T q_super via fp8e4 DoubleRow."""
            q0 = QS * qs
            sT = psT.tile([128, QS], FP32, tag="sT")
            for half in range(2):
                nc.tensor.matmul(
                    sT[:, 512 * half:512 * (half + 1)],
                    lhsT=k8_sb[32 * h:32 * (h + 1), :, 128 * kv:128 * (kv + 1)],
                    rhs=q8_sb[32 * h:32 * (h + 1), :,
                              q0 + 512 * half:q0 + 512 * (half + 1)],
                    start=True, stop=True,
                    perf_mode=mybir.MatmulPerfMode.DoubleRow,
                )
            return sT

        def make_pT(sT, kv, h):
            """p tile: ACT 2*exp(S), or DVE+GPSIMD (1+S)^2 (Taylor tiles).

            Taylor tiles are processed in 512-wide halves so the first pv
            matmuls can start after ~half the ts+tt latency, and the psum
            buf is released by the (cheap) DVE ts rather than the tt.
            """
            if kv in TK[h]:
                pT = ppT_tay.tile([128, QS], BF16_DT, tag="pT")
                t = ptay.tile([128, QS], BF16_DT, tag="tay")
                for hf in (slice(0, 512), slice(512, 1024)):
                    nc.vector.tensor_scalar(
                        out=t[:, hf], in0=sT[:, hf], scalar1=float(EXPSCALE),
                        scalar2=1.0, op0=MULT, op1=ADD)
                    nc.gpsimd.tensor_tensor(
                        out=pT[:, hf], in0=t[:, hf], in1=t[:, hf], op=MULT)
            else:
                pT = ppT_act.tile([128, QS], BF16_DT, tag="pT")
                nc.scalar.activation(
                    out=pT, in_=sT, func=mybir.ActivationFunctionType.Exp,
                    scale=float(EXPSCALE), bias=ln2_sb[:, 0:1])
            return pT

        def acc_slot(accs, h, qt):
            if qt < 7:
                return accs[h], 65 * qt
            return accs[2], 65 * h

        def pv(accs, kv, h, pT, init):
            for qt in range(NQT):
                acc, off = acc_slot(accs, h, qt)
                first_in_bank = qt == 0 or (qt == 7 and h == 0)
                nc.tensor.matmul(
                    acc[:, off:off + 65],
                    lhsT=pT[:, 128 * qt:128 * (qt + 1)],
                    rhs=v_sb[:, kv, 65 * h:65 * (h + 1)],
                    start=(init and kv == 0 and first_in_bank),
                    stop=(kv == NKV - 1),
                    skip_group_check=True,
                )

        def sv_reduce(accC):
            """sv[h] = sum of v over this head's Taylor kv tiles (+count)."""
            items = [(h, kv) for h in (0, 1) for kv in sorted(TK[h])]
            for i, (h, kv) in enumerate(items):
                nc.tensor.matmul(
                    accC[0:1, PSV0 + 65 * h:PSV0 + 65 * (h + 1)],
                    lhsT=ones_col[:, 0:1],
                    rhs=v_sb[:, kv, 65 * h:65 * (h + 1)],
                    start=False, stop=(i == len(items) - 1),
                    skip_group_check=True,
                )
            nc.vector.tensor_copy(out=sv_sb, in_=accC[0:1, PSV0:PSV0 + 130])

        def corrections(accs, h, init):
            """acc[q, :] += sv[h] for every q (K=1 broadcast matmuls)."""
            for qt in range(NQT):
                acc, off = acc_slot(accs, h, qt)
                first_in_bank = qt == 0 or (qt == 7 and h == 0)
                nc.tensor.matmul(
                    acc[:, off:off + 65],
                    lhsT=ones_row[0:1, 0:128],
                    rhs=sv_sb[0:1, 65 * h:65 * (h + 1)],
                    start=(init and first_in_bank), stop=True,
                    skip_group_check=True,
                )

        def norm_mul(accs, h, qts=tuple(range(NQT))):
            """Extract+normalize head h's accumulators into o_sb tiles.

            Only the (cheap) reciprocals and per-qt muls run at the
            superblock boundary; the transposes/copies into oT_sb are
            returned as deferred closures to spread over later iterations.
            """
            accH, accC = accs[h], accs[2]
            rec = pnorm.tile([128, 8], FP32, tag="rec")
            den7 = accH[:, 0:455].rearrange("p (s c) -> p s c", s=7)[:, :, 64]
            nc.vector.reciprocal(rec[:, 0:7], den7)
            nc.vector.reciprocal(rec[:, 7:8],
                                 accC[:, 65 * h + 64:65 * h + 65])
            outs = []
            for qt in qts:
                acc, off = acc_slot(accs, h, qt)
                o_sb = po.tile([128, 64], BF16_DT, tag="o_sb")
                nc.vector.tensor_scalar_mul(
                    out=o_sb, in0=acc[:, off:off + 64],
                    scalar1=rec[:, qt:qt + 1])
                outs.append((qt, o_sb))
            return outs

        def o_transpose(accC, qs, h, qt, o_sb, slot):
            """Transpose one normalized o tile into oT_sb (deferred)."""
            q0 = QS * qs
            trv = accC.bitcast(BF16_DT)
            trp = trv[:, TRB0 + 128 * slot:TRB0 + 128 * (slot + 1)]
            nc.tensor.transpose(trp[0:64, :], o_sb, ident)
            nc.vector.tensor_copy(
                out=oT_sb[0:64, h * N + q0 + 128 * qt:
                          h * N + q0 + 128 * (qt + 1)],
                in_=trp[0:64, :],
            )

        def norm_head(accs, qs, h, qts=tuple(range(NQT))):
            """Boundary norm with inline transposes (tail path)."""
            for i, (qt, o_sb) in enumerate(norm_mul(accs, h, qts)):
                o_transpose(accs[2], qs, h, qt, o_sb, i % 2)

        def outproj_piece(ch, ct, pool=None):
            if pool is None:
                ps = pbig.tile([128, CH], FP32, tag="big", name="ps")
            else:
                ps = pool.tile([128, CH], FP32, tag="sT", name="ps")
            for h in range(2):
                nc.tensor.matmul(
                    ps,
                    lhsT=wo_sb[0:DH, h * C + 128 * ct:h * C + 128 * (ct + 1)],
                    rhs=oT_sb[0:DH, h * N + CH * ch:h * N + CH * (ch + 1)],
                    start=(h == 0), stop=(h == 1),
                )
            st = so.tile([128, CH], BF16_DT, tag="st")
            nc.vector.tensor_scalar_add(
                out=st, in0=ps, scalar1=bo_sb[:, ct:ct + 1])
            nc.sync.dma_start(
                out=poutT[128 * ct:128 * (ct + 1), CH * ch:CH * (ch + 1)],
                in_=st,
            )

        # ---------- startup prefix ----------
        proj_qk(k8_sb, 128, 1, 0, pool=psT)
        proj_qk(q8_sb, 0, 0, 0, pool=psT)
        proj_qk(q8_sb, 0, 0, 1)
        proj_v(0, pool=psT)
        for kv in range(4):
            vtr(kv, pool=psT if kv % 2 else None)

        accs = [pacc.tile([128, 512], FP32, tag=t, name=t)
                for t in ("accA", "accB", "accC")]

        # Filler tasks drip-fed into the attention loop (1 per iteration).
        # The h0 block of qs0 has exactly 32 drains: 28 chunk tasks, q2, q3,
        # sv, and h0's late corrections (which must precede norm h0).
        filler = deque()
        for j in range(1, NCH):
            filler.append(lambda j=j: proj_qk(k8_sb, 128, 1, j))
            filler.append(lambda j=j: proj_v(j))
            filler.append(lambda j=j: (vtr(4 * j), vtr(4 * j + 1)))
            filler.append(lambda j=j: (vtr(4 * j + 2), vtr(4 * j + 3)))
        filler.append(lambda: proj_qk(q8_sb, 0, 0, 2))
        filler.append(lambda: proj_qk(q8_sb, 0, 0, 3))
        filler.append(lambda: sv_reduce(accs[2]))
        filler.append(lambda: corrections(accs, 0, init=False))
        # h1 block of qs0: h1's late corrections, then remaining q chunks
        filler.append(lambda: corrections(accs, 1, init=False))
        for j in range(4, NCH):
            filler.append(lambda j=j: proj_qk(q8_sb, 0, 0, j))

        def drain_filler(nmax):
            for _ in range(min(nmax, len(filler))):
                filler.popleft()()

        # ---------- attention ----------
        # One head per iteration (h0 block then h1 block per superblock):
        # with psT bufs=2 and ONE sT tile per iteration, each s_mm is
        # WAR-decoupled from its buffer's consumer by two full iterations,
        # so ACT/GPSIMD never gate the S^T production ring.
        def next_triple(qs, h, kv):
            if kv + 1 < NKV:
                return (qs, h, kv + 1)
            if h == 0:
                return (qs, 1, 0)
            if qs + 1 < NQS:
                return (qs + 1, 0, 0)
            return None

        sT_next = s_mm(0, 0, 0)
        pT_next = make_pT(sT_next, 0, 0)
        for qs in range(NQS):
            last = qs == NQS - 1
            init = qs == 0
            for h in (0, 1):
                if not init:
                    corrections(accs, h, init=True)
                for kv in range(NKV):
                    pT = pT_next
                    nxt = next_triple(qs, h, kv)
                    if nxt is not None:
                        sT_next = s_mm(nxt[0], nxt[2], nxt[1])
                        pT_next = make_pT(sT_next, nxt[2], nxt[1])
                    else:
                        sT_next = pT_next = None
                    pv(accs, kv, h, pT, init)
                    drain_filler(1)
                if not (last and h == 1):
                    norm_head(accs, qs, h)

            if not last:
                accs = [pacc.tile([128, 512], FP32, tag=t, name=t)
                        for t in ("accA", "accB", "accC")]
                for ch in (2 * qs, 2 * qs + 1):
                    for ct in range(4):
                        filler.append(
                            lambda ch=ch, ct=ct: outproj_piece(ch, ct))
            else:
                # tail: outproj chunk needs only its own 4 qt of both heads
                o1 = norm_mul(accs, 1)
                for i, (qt, o_sb) in enumerate(o1[0:4]):
                    o_transpose(accs[2], qs, 1, qt, o_sb, i % 2)
                for ct in range(4):
                    outproj_piece(2 * qs, ct, pool=psT if ct % 2 else None)
                for i, (qt, o_sb) in enumerate(o1[4:8]):
                    o_transpose(accs[2], qs, 1, qt, o_sb, i % 2)
                for ct in range(4):
                    outproj_piece(2 * qs + 1, ct, pool=psT if ct % 2 else None)
        assert not filler, len(filler)


_NC = None


def _build_nc():
    global _NC
    if _NC is None:
        nc = bacc.Bacc("TRN2", target_bir_lowering=False, debug=False,
                       num_devices=NCORES)
        with tile.TileContext(nc) as tc:
            _emit(tc)
        nc.finalize()
        _NC = nc
    return _NC


def _in_maps(x, w_qkv, b_qkv, w_out, b_out):
    x = np.asarray(x, dtype=np.float32)
    w_qkv = np.asarray(w_qkv, dtype=np.float32)
    b_qkv = np.asarray(b_qkv, dtype=np.float32)
    w_out = np.asarray(w_out, dtype=np.float32)
    b_out = np.asarray(b_out, dtype=np.float32)

    w4 = w_qkv.reshape(C, 3, H, DH)
    b4 = b_qkv.reshape(3, H, DH)
    xT_b = [np.ascontiguousarray(x[b].T).astype(BF16) for b in range(B)]

    maps = []
    for c in range(NCORES):
        b = c // 4
        h0, h1 = 2 * (c % 4), 2 * (c % 4) + 1
        # q/k blocks in [32,2]-split partition order:
        #   [h0 d0-31 | h1 d0-31 | h0 d32-63 | h1 d32-63]
        def qk_block(i):
            return np.concatenate(
                [w4[:, i, h0, 0:32], w4[:, i, h1, 0:32],
                 w4[:, i, h0, 32:64], w4[:, i, h1, 32:64]], axis=1)
        wl = np.concatenate(
            [qk_block(0), qk_block(1), w4[:, 2, h0], w4[:, 2, h1]],
            axis=1).astype(BF16)
        bq = np.zeros((3, 128), np.float32)
        for i, row in ((0, 0), (1, 1)):
            bq[row] = np.concatenate(
                [b4[i, h0, 0:32], b4[i, h1, 0:32],
                 b4[i, h0, 32:64], b4[i, h1, 32:64]])
        bq[2] = np.concatenate([b4[2, h0], b4[2, h1]])
        wo = np.concatenate(
            [w_out[DH * h0:DH * (h0 + 1)], w_out[DH * h1:DH * (h1 + 1)]],
            axis=1).astype(BF16)
        bo = (b_out.reshape(4, 128) if c % 4 == 0
              else np.zeros((4, 128), np.float32))
        maps.append({
            "xT": xT_b[b],
            "wqkv": np.ascontiguousarray(wl),
            "bqkv": bq,
            "wout": np.ascontiguousarray(wo),
            "bout": np.ascontiguousarray(bo.astype(np.float32)),
        })
    return maps


def kernel(x, w_qkv, b_qkv, w_out, b_out, _trace=False, **_trace_kwargs):
    nc = _build_nc()
    maps = _in_maps(x, w_qkv, b_qkv, w_out, b_out)
    res = run_bass_kernel_spmd(nc, maps, core_ids=list(range(NCORES)),
                               trace=_trace, **_trace_kwargs)
    parts = [np.asarray(r["poutT"]).astype(np.float32) for r in res.results]
    out = np.empty((B, N, C), dtype=np.float32)
    for b in range(B):
        acc = parts[4 * b]
        for i in range(1, 4):
            acc = acc + parts[4 * b + i]
        out[b] = acc.T
    if _trace:
        return out, res
    return out


# revision 27
# speedup vs baseline: 1.0066x; 1.0066x over previous
"""Multi-head attention kernel for Trainium2, SPMD over 8 NeuronCores.

Problem: B=2, N=4096, C=512, H=8 heads, DH=64. fp32 I/O.
Sharding: core c -> batch b=c//4, heads {2*(c%4), 2*(c%4)+1}.
Each core computes its 2 heads' attention + a partial output projection
(transposed layout [C, N], bf16); the host sums the 4 partials per batch
and transposes back.

v2 design (ACT was the bottleneck at 267us busy / 327us total):
- S^T matmuls run in fp8e4 DoubleRow (0.5 cyc/row): q,k are projected,
  scaled by 16 and quantized to fp8 in a [32, 2, N] layout (contraction
  DH=64 split as Ki=32 partitions x Ko=2), halving the dominant PE term.
- ~31% of the exp tiles are offloaded from ACT: softmax weights for
  those (kv, head) tiles use the 2nd-order Taylor form
  (1+S)^2 + 1 = 2*exp(S) + O(S^3)  (S has std ~0.07, |S|<0.65)
  computed as one DVE tensor_scalar (t = 1 + S, reading PSUM) plus one
  GPSIMD tensor_tensor (t*t). ACT tiles compute 2*exp(S) via bias=ln2,
  and the "+1" constant is folded in algebraically: a per-head
  sum-of-v over the Taylor kv tiles (sv) is accumulated into the o/den
  accumulators with K=1 broadcast matmuls, so normalization divides the
  consistent 2x-scaled numerator/denominator.
- next-iteration pT producers are emitted before the boundary norm work
  so ACT/DVE/GPSIMD never wait on the norm chain; norm uses batched
  reciprocals and ping-pong transpose slots carved out of accC's free
  PSUM space.
- startup: PE-warmup transpose chain (p-state ramp), x chunk0 + the
  q/k halves of w are DMA'd first across 3 queues.
"""

import math
from collections import deque

import numpy as np
import ml_dtypes

import concourse.tile as tile
from concourse import bacc, mybir
from concourse.bass_utils import run_bass_kernel_spmd
from concourse.masks import make_identity

BF16 = ml_dtypes.bfloat16
E4M3 = ml_dtypes.float8_e4m3fn

B, N, C, H = 2, 4096, 512, 8
DH = C // H          # 64
NCORES = 8
SCALE = C ** -0.5    # reference scales by hidden_dim, not head_dim
PRE = 16.0           # q/k prescale into fp8e4 range
EXPSCALE = SCALE / (PRE * PRE)
LN2 = math.log(2.0)

QS = 1024            # query superblock
NQS = N // QS        # 4
NKV = N // 128       # 32 kv tiles
NQT = QS // 128      # 8 query tiles per superblock
CH = 512             # token chunk for projections
NCH = N // CH        # 8

FP32 = mybir.dt.float32
BF16_DT = mybir.dt.bfloat16
FP8 = mybir.dt.float8e4

# Taylor (DVE+GPSIMD) kv tiles per head; same sets for every superblock.
TK = (frozenset(range(0, 30, 3)), frozenset(range(1, 31, 3)))  # 10 + 10

PSV0 = 136     # fp32 col in accC where the sv accumulator lives [136:266)
TRB0 = 544     # bf16 col in accC for transpose ping-pong slots (2x128)
WARMUP = 40    # PE p-state warmup transposes

ADD = mybir.AluOpType.add
MULT = mybir.AluOpType.mult


def _emit(tc):
    nc = tc.nc
    xT = nc.dram_tensor("xT", [C, N], BF16_DT, kind="ExternalInput").ap()
    wqkv = nc.dram_tensor("wqkv", [C, 6 * DH], BF16_DT, kind="ExternalInput").ap()
    bqkv = nc.dram_tensor("bqkv", [3, 128], FP32, kind="ExternalInput").ap()
    wout = nc.dram_tensor("wout", [DH, 2 * C], BF16_DT, kind="ExternalInput").ap()
    bout = nc.dram_tensor("bout", [4, 128], FP32, kind="ExternalInput").ap()
    poutT = nc.dram_tensor("poutT", [C, N], BF16_DT, kind="ExternalOutput").ap()

    with (
        tc.tile_pool(name="singles", bufs=1) as singles,
        tc.tile_pool(name="psum_big", bufs=1, space="PSUM") as pbig,
        tc.tile_pool(name="psum_sT", bufs=2, space="PSUM") as psT,
        tc.tile_pool(name="psum_acc", bufs=1, space="PSUM") as pacc,
        # separate pT pools per producer engine: a shared pool would create
        # WAW buffer deps chaining ACT behind the GPSIMD tt stream
        tc.tile_pool(name="pT_act", bufs=5) as ppT_act,
        tc.tile_pool(name="pT_tay", bufs=4) as ppT_tay,
        tc.tile_pool(name="tay_pool", bufs=6) as ptay,
        tc.tile_pool(name="q8_pool", bufs=3) as pq8,
        tc.tile_pool(name="norm_pool", bufs=4) as pnorm,
        tc.tile_pool(name="o_pool", bufs=18) as po,
        tc.tile_pool(name="stage_out", bufs=4) as so,
    ):
        # --- resident SBUF tensors ---
        xT_sb = singles.tile([128, 4, N], BF16_DT)
        w_sb = singles.tile([128, 4, 6 * DH], BF16_DT)
        bq_sb = singles.tile([128, 3], FP32)
        wo_sb = singles.tile([128, 2 * C], BF16_DT)
        bo_sb = singles.tile([128, 4], FP32)
        ident = singles.tile([128, 128], BF16_DT)
        ones_col = singles.tile([128, 1], BF16_DT)
        ones_row = singles.tile([1, 128], FP32)
        # q/k fp8 [32,2]-split: parts 0-31 h0, 32-63 h1; free (j, h... )
        # element (p, j, n) = PRE * q_{d = 32j + p}(n), head = p//32
        q8_sb = singles.tile([64, 2, N], FP8)
        k8_sb = singles.tile([64, 2, N], FP8)
        vT_sb = singles.tile([128, N], BF16_DT)
        v_sb = singles.tile([128, NKV, 130], BF16_DT)
        oT_sb = singles.tile([64, 2 * N], BF16_DT)
        sv_sb = singles.tile([1, 130], FP32)
        ln2_sb = singles.tile([128, 1], FP32)
        warm = singles.tile([128, 1], FP32)

        make_identity(nc, ident)
        nc.vector.memset(ones_col, 1.0)
        nc.vector.memset(ones_row, 1.0)
        nc.vector.memset(ln2_sb, float(LN2))
        nc.vector.memset(v_sb[:, :, 64:65], 1.0)
        nc.vector.memset(v_sb[:, :, 129:130], 1.0)
        nc.vector.memset(warm, 0.0)
        # load the Exp table set during setup
        nc.scalar.activation(out=warm, in_=warm,
                             func=mybir.ActivationFunctionType.Exp)
        # PE p-state warmup: keep PE continuously busy through the DMA wait
        for _ in range(WARMUP):
            wt = pbig.tile([128, 128], BF16_DT, tag="big", name="warm")
            nc.tensor.transpose(wt, ident, ident)

        # --- DMA schedule: x ch0 + w(qk) first, on 3 HWDGE queues ---
        QQ = (nc.sync, nc.scalar, nc.gpsimd)

        def x_piece(ch, kt):
            # sync/scalar HWDGE only: gpsimd's DMA issue occupies the Pool
            # engine, which the Taylor tt stream needs
            QQ[(4 * ch + kt) % 2].dma_start(
                out=xT_sb[:, kt, CH * ch:CH * (ch + 1)],
                in_=xT[128 * kt:128 * (kt + 1), CH * ch:CH * (ch + 1)])

        for kt in range(4):
            x_piece(0, kt)
        for kt in range(4):
            (nc.sync if kt % 2 else nc.scalar).dma_start(
                out=w_sb[:, kt, 0:256], in_=wqkv[128 * kt:128 * (kt + 1), 0:256])
        for j in range(3):
            nc.scalar.dma_start(out=bq_sb[:, j:j + 1], in_=bqkv[j, :])
        for kt in range(4):
            x_piece(1, kt)
        for kt in range(4):
            (nc.sync if kt % 2 else nc.scalar).dma_start(
                out=w_sb[:, kt, 256:384],
                in_=wqkv[128 * kt:128 * (kt + 1), 256:384])
        nc.sync.dma_start(out=wo_sb[0:DH, :], in_=wout[:, :])
        for j in range(4):
            nc.scalar.dma_start(out=bo_sb[:, j:j + 1], in_=bout[j, :])
        for ch in range(2, NCH):
            for kt in range(4):
                x_piece(ch, kt)

        # ---------- emission helpers ----------

        def proj_qk(dst8, wcol0, bcol, ch, pool=None):
            """Project one 512-token chunk of q or k into fp8 [32,2] layout."""
            sl = slice(CH * ch, CH * (ch + 1))
            if pool is None:
                ps = pbig.tile([128, CH], FP32, tag="big", name="ps")
            else:
                ps = pool.tile([128, CH], FP32, tag="sT", name="ps")
            for kt in range(4):
                nc.tensor.matmul(
                    ps, lhsT=w_sb[:, kt, wcol0:wcol0 + 128],
                    rhs=xT_sb[:, kt, sl], start=(kt == 0), stop=(kt == 3))
            # rows 0-63 = j0 (both heads): convert in place
            nc.vector.tensor_scalar(
                out=dst8[0:64, 0, sl], in0=ps[0:64, :],
                scalar1=bq_sb[0:64, bcol:bcol + 1], scalar2=float(PRE),
                op0=ADD, op1=MULT)
            # rows 64-127 = j1: convert, then partition-shift via DMA
            qt8 = pq8.tile([128, CH], FP8, tag="q8")
            nc.vector.tensor_scalar(
                out=qt8[64:128, :], in0=ps[64:128, :],
                scalar1=bq_sb[64:128, bcol:bcol + 1], scalar2=float(PRE),
                op0=ADD, op1=MULT)
            # sync queue, NOT scalar: a DMA waiting on its ts input would
            # stall every exp behind it in the ACT queue
            nc.sync.dma_start(out=dst8[0:64, 1, sl], in_=qt8[64:128, :])

        def proj_v(ch, pool=None):
            sl = slice(CH * ch, CH * (ch + 1))
            if pool is None:
                ps = pbig.tile([128, CH], FP32, tag="big", name="ps")
            else:
                ps = pool.tile([128, CH], FP32, tag="sT", name="ps")
            for kt in range(4):
                nc.tensor.matmul(
                    ps, lhsT=w_sb[:, kt, 256:384],
                    rhs=xT_sb[:, kt, sl], start=(kt == 0), stop=(kt == 3))
            nc.vector.tensor_scalar_add(
                out=vT_sb[:, sl], in0=ps, scalar1=bq_sb[:, 2:3])

        def vtr(kv, pool=None):
            """Transpose v^T tile kv into v_sb [tok, d] layout."""
            if pool is None:
                trp = pbig.tile([128, 128], BF16_DT, tag="big", name="trp")
            else:
                trp = pool.tile([128, 128], BF16_DT, tag="sT", name="trp")
            nc.tensor.transpose(trp, vT_sb[:, 128 * kv:128 * (kv + 1)], ident)
            src = trp.rearrange("p (j c) -> p j c", j=2)
            dst = v_sb[:, kv, 0:130].rearrange("p (j c) -> p j c", j=2)
            nc.vector.tensor_copy(out=dst[:, :, 0:64], in_=src)

        def s_mm(qs, kv, h):
            """S^T = k_tile^T q_super via fp8e4 DoubleRow."""
            q0 = QS * qs
            sT = psT.tile([128, QS], FP32, tag="sT")
            for half in range(2):
                nc.tensor.matmul(
                    sT[:, 512 * half:512 * (half + 1)],
                    lhsT=k8_sb[32 * h:32 * (h + 1), :, 128 * kv:128 * (kv + 1)],
                    rhs=q8_sb[32 * h:32 * (h + 1), :,
                              q0 + 512 * half:q0 + 512 * (half + 1)],
                    start=True, stop=True,
                    perf_mode=mybir.MatmulPerfMode.DoubleRow,
                )
            return sT

        def make_pT(sT, kv, h):
            """p tile: ACT 2*exp(S), or DVE+GPSIMD (1+S)^2 (Taylor tiles).

            Taylor tiles are processed in 512-wide halves so the first pv
            matmuls can start after ~half the ts+tt latency, and the psum
            buf is released by the (cheap) DVE ts rather than the tt.
            """
            if kv in TK[h]:
                pT = ppT_tay.tile([128, QS], BF16_DT, tag="pT")
                t = ptay.tile([128, QS], BF16_DT, tag="tay")
                for hf in (slice(0, 512), slice(512, 1024)):
                    nc.vector.tensor_scalar(
                        out=t[:, hf], in0=sT[:, hf], scalar1=float(EXPSCALE),
                        scalar2=1.0, op0=MULT, op1=ADD)
                    nc.gpsimd.tensor_tensor(
                        out=pT[:, hf], in0=t[:, hf], in1=t[:, hf], op=MULT)
            else:
                pT = ppT_act.tile([128, QS], BF16_DT, tag="pT")
                nc.scalar.activation(
                    out=pT, in_=sT, func=mybir.ActivationFunctionType.Exp,
                    scale=float(EXPSCALE), bias=ln2_sb[:, 0:1])
            return pT

        def acc_slot(accs, h, qt):
            if qt < 7:
                return accs[h], 65 * qt
            return accs[2], 65 * h

        def pv(accs, kv, h, pT, init):
            for qt in range(NQT):
                acc, off = acc_slot(accs, h, qt)
                first_in_bank = qt == 0 or (qt == 7 and h == 0)
                nc.tensor.matmul(
                    acc[:, off:off + 65],
                    lhsT=pT[:, 128 * qt:128 * (qt + 1)],
                    rhs=v_sb[:, kv, 65 * h:65 * (h + 1)],
                    start=(init and kv == 0 and first_in_bank),
                    stop=(kv == NKV - 1),
                    skip_group_check=True,
                )

        def sv_reduce(accC):
            """sv[h] = sum of v over this head's Taylor kv tiles (+count)."""
            items = [(h, kv) for h in (0, 1) for kv in sorted(TK[h])]
            for i, (h, kv) in enumerate(items):
                nc.tensor.matmul(
                    accC[0:1, PSV0 + 65 * h:PSV0 + 65 * (h + 1)],
                    lhsT=ones_col[:, 0:1],
                    rhs=v_sb[:, kv, 65 * h:65 * (h + 1)],
                    start=False, stop=(i == len(items) - 1),
                    skip_group_check=True,
                )
            nc.vector.tensor_copy(out=sv_sb, in_=accC[0:1, PSV0:PSV0 + 130])

        def corrections(accs, h, init):
            """acc[q, :] += sv[h] for every q (K=1 broadcast matmuls)."""
            for qt in range(NQT):
                acc, off = acc_slot(accs, h, qt)
                first_in_bank = qt == 0 or (qt == 7 and h == 0)
                nc.tensor.matmul(
                    acc[:, off:off + 65],
                    lhsT=ones_row[0:1, 0:128],
                    rhs=sv_sb[0:1, 65 * h:65 * (h + 1)],
                    start=(init and first_in_bank), stop=True,
                    skip_group_check=True,
                )

        def norm_mul(accs, h, qts=tuple(range(NQT))):
            """Extract+normalize head h's accumulators into o_sb tiles.

            Only the (cheap) reciprocals and per-qt muls run at the
            superblock boundary; the transposes/copies into oT_sb are
            returned as deferred closures to spread over later iterations.
            """
            accH, accC = accs[h], accs[2]
            rec = pnorm.tile([128, 8], FP32, tag="rec")
            den7 = accH[:, 0:455].rearrange("p (s c) -> p s c", s=7)[:, :, 64]
            nc.vector.reciprocal(rec[:, 0:7], den7)
            nc.vector.reciprocal(rec[:, 7:8],
                                 accC[:, 65 * h + 64:65 * h + 65])
            outs = []
            for qt in qts:
                acc, off = acc_slot(accs, h, qt)
                o_sb = po.tile([128, 64], BF16_DT, tag="o_sb")
                nc.vector.tensor_scalar_mul(
                    out=o_sb, in0=acc[:, off:off + 64],
                    scalar1=rec[:, qt:qt + 1])
                outs.append((qt, o_sb))
            return outs

        def o_transpose(accC, qs, h, qt, o_sb, slot):
            """Transpose one normalized o tile into oT_sb (deferred)."""
            q0 = QS * qs
            trv = accC.bitcast(BF16_DT)
            trp = trv[:, TRB0 + 128 * slot:TRB0 + 128 * (slot + 1)]
            nc.tensor.transpose(trp[0:64, :], o_sb, ident)
            nc.vector.tensor_copy(
                out=oT_sb[0:64, h * N + q0 + 128 * qt:
                          h * N + q0 + 128 * (qt + 1)],
                in_=trp[0:64, :],
            )

        def norm_head(accs, qs, h, qts=tuple(range(NQT))):
            """Boundary norm with inline transposes (tail path)."""
            for i, (qt, o_sb) in enumerate(norm_mul(accs, h, qts)):
                o_transpose(accs[2], qs, h, qt, o_sb, i % 2)

        def outproj_piece(ch, ct, pool=None):
            if pool is None:
                ps = pbig.tile([128, CH], FP32, tag="big", name="ps")
            else:
                ps = pool.tile([128, CH], FP32, tag="sT", name="ps")
            for h in range(2):
                nc.tensor.matmul(
                    ps,
                    lhsT=wo_sb[0:DH, h * C + 128 * ct:h * C + 128 * (ct + 1)],
                    rhs=oT_sb[0:DH, h * N + CH * ch:h * N + CH * (ch + 1)],
                    start=(h == 0), stop=(h == 1),
                )
            st = so.tile([128, CH], BF16_DT, tag="st")
            nc.vector.tensor_scalar_add(
                out=st, in0=ps, scalar1=bo_sb[:, ct:ct + 1])
            nc.sync.dma_start(
                out=poutT[128 * ct:128 * (ct + 1), CH * ch:CH * (ch + 1)],
                in_=st,
            )

        # ---------- startup prefix ----------
        proj_qk(k8_sb, 128, 1, 0, pool=psT)
        proj_qk(q8_sb, 0, 0, 0, pool=psT)
        proj_qk(q8_sb, 0, 0, 1)
        proj_v(0, pool=psT)
        for kv in range(4):
            vtr(kv, pool=psT if kv % 2 else None)

        accs = [pacc.tile([128, 512], FP32, tag=t, name=t)
                for t in ("accA", "accB", "accC")]

        # Filler tasks drip-fed into the attention loop (1 per iteration).
        # The h0 block of qs0 has exactly 32 drains: 28 chunk tasks, q2, q3,
        # sv, and h0's late corrections (which must precede norm h0).
        filler = deque()
        for j in range(1, NCH):
            filler.append(lambda j=j: proj_qk(k8_sb, 128, 1, j))
            filler.append(lambda j=j: proj_v(j))
            filler.append(lambda j=j: (vtr(4 * j), vtr(4 * j + 1)))
            filler.append(lambda j=j: (vtr(4 * j + 2), vtr(4 * j + 3)))
        filler.append(lambda: proj_qk(q8_sb, 0, 0, 2))
        filler.append(lambda: proj_qk(q8_sb, 0, 0, 3))
        filler.append(lambda: sv_reduce(accs[2]))
        filler.append(lambda: corrections(accs, 0, init=False))
        # h1 block of qs0: h1's late corrections, then remaining q chunks
        filler.append(lambda: corrections(accs, 1, init=False))
        for j in range(4, NCH):
            filler.append(lambda j=j: proj_qk(q8_sb, 0, 0, j))

        def drain_filler(nmax):
            for _ in range(min(nmax, len(filler))):
                filler.popleft()()

        # ---------- attention ----------
        # One head per iteration (h0 block then h1 block per superblock):
        # with psT bufs=2 and ONE sT tile per iteration, each s_mm is
        # WAR-decoupled from its buffer's consumer by two full iterations,
        # so ACT/GPSIMD never gate the S^T production ring.
        def next_triple(qs, h, kv):
            if kv + 1 < NKV:
                return (qs, h, kv + 1)
            if h == 0:
                return (qs, 1, 0)
            if qs + 1 < NQS:
                return (qs + 1, 0, 0)
            return None

        sT_next = s_mm(0, 0, 0)
        pT_next = make_pT(sT_next, 0, 0)
        for qs in range(NQS):
            last = qs == NQS - 1
            init = qs == 0
            for h in (0, 1):
                if not init:
                    corrections(accs, h, init=True)
                for kv in range(NKV):
                    pT = pT_next
                    nxt = next_triple(qs, h, kv)
                    if nxt is not None:
                        sT_next = s_mm(nxt[0], nxt[2], nxt[1])
                        pT_next = make_pT(sT_next, nxt[2], nxt[1])
                    else:
                        sT_next = pT_next = None
                    pv(accs, kv, h, pT, init)
                    drain_filler(1)
                if not (last and h == 1):
                    norm_head(accs, qs, h)

            if not last:
                accs = [pacc.tile([128, 512], FP32, tag=t, name=t)
                        for t in ("accA", "accB", "accC")]
                for ch in (2 * qs, 2 * qs + 1):
                    for ct in range(4):
                        filler.append(
                            lambda ch=ch, ct=ct: outproj_piece(ch, ct))
            else:
                # tail: outproj chunk needs only its own 4 qt of both heads
                o1 = norm_mul(accs, 1)
                for i, (qt, o_sb) in enumerate(o1[0:4]):
                    o_transpose(accs[2], qs, 1, qt, o_sb, i % 2)
                for ct in range(4):
                    outproj_piece(2 * qs, ct, pool=psT if ct % 2 else None)
                for i, (qt, o_sb) in enumerate(o1[4:8]):
                    o_transpose(accs[2], qs, 1, qt, o_sb, i % 2)
                for ct in range(4):
                    outproj_piece(2 * qs + 1, ct, pool=psT if ct % 2 else None)
        assert not filler, len(filler)


_NC = None


def _build_nc():
    global _NC
    if _NC is None:
        nc = bacc.Bacc("TRN2", target_bir_lowering=False, debug=False,
                       num_devices=NCORES)
        with tile.TileContext(nc) as tc:
            _emit(tc)
        nc.finalize()
        _NC = nc
    return _NC


def _in_maps(x, w_qkv, b_qkv, w_out, b_out):
    x = np.asarray(x, dtype=np.float32)
    w_qkv = np.asarray(w_qkv, dtype=np.float32)
    b_qkv = np.asarray(b_qkv, dtype=np.float32)
    w_out = np.asarray(w_out, dtype=np.float32)
    b_out = np.asarray(b_out, dtype=np.float32)

    w4 = w_qkv.reshape(C, 3, H, DH)
    b4 = b_qkv.reshape(3, H, DH)
    xT_b = [np.ascontiguousarray(x[b].T).astype(BF16) for b in range(B)]

    maps = []
    for c in range(NCORES):
        b = c // 4
        h0, h1 = 2 * (c % 4), 2 * (c % 4) + 1
        # q/k blocks in [32,2]-split partition order:
        #   [h0 d0-31 | h1 d0-31 | h0 d32-63 | h1 d32-63]
        def qk_block(i):
            return np.concatenate(
                [w4[:, i, h0, 0:32], w4[:, i, h1, 0:32],
                 w4[:, i, h0, 32:64], w4[:, i, h1, 32:64]], axis=1)
        wl = np.concatenate(
            [qk_block(0), qk_block(1), w4[:, 2, h0], w4[:, 2, h1]],
            axis=1).astype(BF16)
        bq = np.zeros((3, 128), np.float32)
        for i, row in ((0, 0), (1, 1)):
            bq[row] = np.concatenate(
                [b4[i, h0, 0:32], b4[i, h1, 0:32],
                 b4[i, h0, 32:64], b4[i, h1, 32:64]])
        bq[2] = np.concatenate([b4[2, h0], b4[2, h1]])
        wo = np.concatenate(
            [w_out[DH * h0:DH * (h0 + 1)], w_out[DH * h1:DH * (h1 + 1)]],
            axis=1).astype(BF16)
        bo = (b_out.reshape(4, 128) if c % 4 == 0
              else np.zeros((4, 128), np.float32))
        maps.append({
            "xT": xT_b[b],
            "wqkv": np.ascontiguousarray(wl),
            "bqkv": bq,
            "wout": np.ascontiguousarray(wo),
            "bout": np.ascontiguousarray(bo.astype(np.float32)),
        })
    return maps


def kernel(x, w_qkv, b_qkv, w_out, b_out, _trace=False, **_trace_kwargs):
    nc = _build_nc()
    maps = _in_maps(x, w_qkv, b_qkv, w_out, b_out)
    res = run_bass_kernel_spmd(nc, maps, core_ids=list(range(NCORES)),
                               trace=_trace, **_trace_kwargs)
    parts = [np.asarray(r["poutT"]).astype(np.float32) for r in res.results]
    out = np.empty((B, N, C), dtype=np.float32)
    for b in range(B):
        acc = parts[4 * b]
        for i in range(1, 4):
            acc = acc + parts[4 * b + i]
        out[b] = acc.T
    if _trace:
        return out, res
    return out


# revision 31
# speedup vs baseline: 1.0089x; 1.0022x over previous
"""Multi-head attention kernel for Trainium2, SPMD over 8 NeuronCores.

Problem: B=2, N=4096, C=512, H=8 heads, DH=64. fp32 I/O.
Sharding: core c -> batch b=c//4, heads {2*(c%4), 2*(c%4)+1}.
Each core computes its 2 heads' attention + a partial output projection
(transposed layout [C, N], bf16); the host sums the 4 partials per batch
and transposes back.

v2 design (ACT was the bottleneck at 267us busy / 327us total):
- S^T matmuls run in fp8e4 DoubleRow (0.5 cyc/row): q,k are projected,
  scaled by 16 and quantized to fp8 in a [32, 2, N] layout (contraction
  DH=64 split as Ki=32 partitions x Ko=2), halving the dominant PE term.
- ~31% of the exp tiles are offloaded from ACT: softmax weights for
  those (kv, head) tiles use the 2nd-order Taylor form
  (1+S)^2 + 1 = 2*exp(S) + O(S^3)  (S has std ~0.07, |S|<0.65)
  computed as one DVE tensor_scalar (t = 1 + S, reading PSUM) plus one
  GPSIMD tensor_tensor (t*t). ACT tiles compute 2*exp(S) via bias=ln2,
  and the "+1" constant is folded in algebraically: a per-head
  sum-of-v over the Taylor kv tiles (sv) is accumulated into the o/den
  accumulators with K=1 broadcast matmuls, so normalization divides the
  consistent 2x-scaled numerator/denominator.
- next-iteration pT producers are emitted before the boundary norm work
  so ACT/DVE/GPSIMD never wait on the norm chain; norm uses batched
  reciprocals and ping-pong transpose slots carved out of accC's free
  PSUM space.
- startup: PE-warmup transpose chain (p-state ramp), x chunk0 + the
  q/k halves of w are DMA'd first across 3 queues.
"""

import math
from collections import deque

import numpy as np
import ml_dtypes

import concourse.tile as tile
from concourse import bacc, mybir
from concourse.bass_utils import run_bass_kernel_spmd
from concourse.masks import make_identity

BF16 = ml_dtypes.bfloat16
E4M3 = ml_dtypes.float8_e4m3fn

B, N, C, H = 2, 4096, 512, 8
DH = C // H          # 64
NCORES = 8
SCALE = C ** -0.5    # reference scales by hidden_dim, not head_dim
PRE = 16.0           # q/k prescale into fp8e4 range
EXPSCALE = SCALE / (PRE * PRE)
LN2 = math.log(2.0)

QS = 1024            # query superblock
NQS = N // QS        # 4
NKV = N // 128       # 32 kv tiles
NQT = QS // 128      # 8 query tiles per superblock
CH = 512             # token chunk for projections
NCH = N // CH        # 8

FP32 = mybir.dt.float32
BF16_DT = mybir.dt.bfloat16
FP8 = mybir.dt.float8e4

# Taylor (DVE+GPSIMD) kv tiles per head; same sets for every superblock.
TK = (frozenset(range(0, 30, 3)), frozenset(range(1, 31, 3)))  # 10 + 10

PSV0 = 136     # fp32 col in accC where the sv accumulator lives [136:266)
TRB0 = 544     # bf16 col in accC for transpose ping-pong slots (2x128)
WARMUP = 40    # PE p-state warmup transposes

ADD = mybir.AluOpType.add
MULT = mybir.AluOpType.mult


def _emit(tc):
    nc = tc.nc
    xT = nc.dram_tensor("xT", [C, N], BF16_DT, kind="ExternalInput").ap()
    wqkv = nc.dram_tensor("wqkv", [C, 6 * DH], BF16_DT, kind="ExternalInput").ap()
    bqkv = nc.dram_tensor("bqkv", [3, 128], FP32, kind="ExternalInput").ap()
    wout = nc.dram_tensor("wout", [DH, 2 * C], BF16_DT, kind="ExternalInput").ap()
    bout = nc.dram_tensor("bout", [4, 128], FP32, kind="ExternalInput").ap()
    poutT = nc.dram_tensor("poutT", [C, N], BF16_DT, kind="ExternalOutput").ap()

    with (
        tc.tile_pool(name="singles", bufs=1) as singles,
        tc.tile_pool(name="psum_big", bufs=1, space="PSUM") as pbig,
        tc.tile_pool(name="psum_sT", bufs=2, space="PSUM") as psT,
        tc.tile_pool(name="psum_acc", bufs=1, space="PSUM") as pacc,
        # separate pT pools per producer engine: a shared pool would create
        # WAW buffer deps chaining ACT behind the GPSIMD tt stream
        tc.tile_pool(name="pT_act", bufs=5) as ppT_act,
        tc.tile_pool(name="pT_tay", bufs=4) as ppT_tay,
        tc.tile_pool(name="tay_pool", bufs=6) as ptay,
        tc.tile_pool(name="q8_pool", bufs=3) as pq8,
        tc.tile_pool(name="norm_pool", bufs=4) as pnorm,
        tc.tile_pool(name="o_pool", bufs=18) as po,
        tc.tile_pool(name="stage_out", bufs=4) as so,
    ):
        # --- resident SBUF tensors ---
        xT_sb = singles.tile([128, 4, N], BF16_DT)
        w_sb = singles.tile([128, 4, 6 * DH], BF16_DT)
        bq_sb = singles.tile([128, 3], FP32)
        wo_sb = singles.tile([128, 2 * C], BF16_DT)
        bo_sb = singles.tile([128, 4], FP32)
        ident = singles.tile([128, 128], BF16_DT)
        ones_col = singles.tile([128, 1], BF16_DT)
        ones_row = singles.tile([1, 128], FP32)
        # q/k fp8 [32,2]-split: parts 0-31 h0, 32-63 h1; free (j, h... )
        # element (p, j, n) = PRE * q_{d = 32j + p}(n), head = p//32
        q8_sb = singles.tile([64, 2, N], FP8)
        k8_sb = singles.tile([64, 2, N], FP8)
        vT_sb = singles.tile([128, N], BF16_DT)
        v_sb = singles.tile([128, NKV, 130], BF16_DT)
        oT_sb = singles.tile([64, 2 * N], BF16_DT)
        sv_sb = singles.tile([1, 130], FP32)
        ln2_sb = singles.tile([128, 1], FP32)
        warm = singles.tile([128, 1], FP32)

        make_identity(nc, ident)
        nc.vector.memset(ones_col, 1.0)
        nc.vector.memset(ones_row, 1.0)
        nc.vector.memset(ln2_sb, float(LN2))
        nc.vector.memset(v_sb[:, :, 64:65], 1.0)
        nc.vector.memset(v_sb[:, :, 129:130], 1.0)
        nc.vector.memset(warm, 0.0)
        # load the Exp table set during setup
        nc.scalar.activation(out=warm, in_=warm,
                             func=mybir.ActivationFunctionType.Exp)
        # PE p-state warmup: keep PE continuously busy through the DMA wait
        for _ in range(WARMUP):
            wt = pbig.tile([128, 128], BF16_DT, tag="big", name="warm")
            nc.tensor.transpose(wt, ident, ident)

        # --- DMA schedule: x ch0 + w(qk) first, on 3 HWDGE queues ---
        QQ = (nc.sync, nc.scalar, nc.gpsimd)

        def x_piece(ch, kt):
            # sync/scalar HWDGE only: gpsimd's DMA issue occupies the Pool
            # engine, which the Taylor tt stream needs
            QQ[(4 * ch + kt) % 2].dma_start(
                out=xT_sb[:, kt, CH * ch:CH * (ch + 1)],
                in_=xT[128 * kt:128 * (kt + 1), CH * ch:CH * (ch + 1)])

        for kt in range(4):
            x_piece(0, kt)
        for kt in range(4):
            (nc.sync if kt % 2 else nc.scalar).dma_start(
                out=w_sb[:, kt, 0:256], in_=wqkv[128 * kt:128 * (kt + 1), 0:256])
        for j in range(3):
            nc.scalar.dma_start(out=bq_sb[:, j:j + 1], in_=bqkv[j, :])
        for kt in range(4):
            x_piece(1, kt)
        for kt in range(4):
            (nc.sync if kt % 2 else nc.scalar).dma_start(
                out=w_sb[:, kt, 256:384],
                in_=wqkv[128 * kt:128 * (kt + 1), 256:384])
        nc.sync.dma_start(out=wo_sb[0:DH, :], in_=wout[:, :])
        for j in range(4):
            nc.scalar.dma_start(out=bo_sb[:, j:j + 1], in_=bout[j, :])
        for ch in range(2, NCH):
            for kt in range(4):
                x_piece(ch, kt)

        # ---------- emission helpers ----------

        def proj_qk(dst8, wcol0, bcol, ch, pool=None):
            """Project one 512-token chunk of q or k into fp8 [32,2] layout."""
            sl = slice(CH * ch, CH * (ch + 1))
            if pool is None:
                ps = pbig.tile([128, CH], FP32, tag="big", name="ps")
            else:
                ps = pool.tile([128, CH], FP32, tag="sT", name="ps")
            for kt in range(4):
                nc.tensor.matmul(
                    ps, lhsT=w_sb[:, kt, wcol0:wcol0 + 128],
                    rhs=xT_sb[:, kt, sl], start=(kt == 0), stop=(kt == 3))
            # rows 0-63 = j0 (both heads): convert in place
            nc.vector.tensor_scalar(
                out=dst8[0:64, 0, sl], in0=ps[0:64, :],
                scalar1=bq_sb[0:64, bcol:bcol + 1], scalar2=float(PRE),
                op0=ADD, op1=MULT)
            # rows 64-127 = j1: convert, then partition-shift via DMA
            qt8 = pq8.tile([128, CH], FP8, tag="q8")
            nc.vector.tensor_scalar(
                out=qt8[64:128, :], in0=ps[64:128, :],
                scalar1=bq_sb[64:128, bcol:bcol + 1], scalar2=float(PRE),
                op0=ADD, op1=MULT)
            # sync queue, NOT scalar: a DMA waiting on its ts input would
            # stall every exp behind it in the ACT queue
            nc.sync.dma_start(out=dst8[0:64, 1, sl], in_=qt8[64:128, :])

        def proj_v(ch, pool=None):
            sl = slice(CH * ch, CH * (ch + 1))
            if pool is None:
                ps = pbig.tile([128, CH], FP32, tag="big", name="ps")
            else:
                ps = pool.tile([128, CH], FP32, tag="sT", name="ps")
            for kt in range(4):
                nc.tensor.matmul(
                    ps, lhsT=w_sb[:, kt, 256:384],
                    rhs=xT_sb[:, kt, sl], start=(kt == 0), stop=(kt == 3))
            nc.vector.tensor_scalar_add(
                out=vT_sb[:, sl], in0=ps, scalar1=bq_sb[:, 2:3])

        def vtr(kv, pool=None):
            """Transpose v^T tile kv into v_sb [tok, d] layout."""
            if pool is None:
                trp = pbig.tile([128, 128], BF16_DT, tag="big", name="trp")
            else:
                trp = pool.tile([128, 128], BF16_DT, tag="sT", name="trp")
            nc.tensor.transpose(trp, vT_sb[:, 128 * kv:128 * (kv + 1)], ident)
            src = trp.rearrange("p (j c) -> p j c", j=2)
            dst = v_sb[:, kv, 0:130].rearrange("p (j c) -> p j c", j=2)
            nc.vector.tensor_copy(out=dst[:, :, 0:64], in_=src)

        def s_mm(qs, kv, h):
            """S^T = k_tile^T q_super via fp8e4 DoubleRow."""
            q0 = QS * qs
            sT = psT.tile([128, QS], FP32, tag="sT")
            for half in range(2):
                nc.tensor.matmul(
                    sT[:, 512 * half:512 * (half + 1)],
                    lhsT=k8_sb[32 * h:32 * (h + 1), :, 128 * kv:128 * (kv + 1)],
                    rhs=q8_sb[32 * h:32 * (h + 1), :,
                              q0 + 512 * half:q0 + 512 * (half + 1)],
                    start=True, stop=True,
                    perf_mode=mybir.MatmulPerfMode.DoubleRow,
                )
            return sT

        def make_pT(sT, kv, h):
            """p tile: ACT 2*exp(S), or DVE+GPSIMD (1+S)^2 (Taylor tiles).

            Taylor tiles are processed in 512-wide halves so the first pv
            matmuls can start after ~half the ts+tt latency, and the psum
            buf is released by the (cheap) DVE ts rather than the tt.
            """
            if kv in TK[h]:
                pT = ppT_tay.tile([128, QS], BF16_DT, tag="pT")
                t = ptay.tile([128, QS], BF16_DT, tag="tay")
                for hf in (slice(0, 512), slice(512, 1024)):
                    nc.vector.tensor_scalar(
                        out=t[:, hf], in0=sT[:, hf], scalar1=float(EXPSCALE),
                        scalar2=1.0, op0=MULT, op1=ADD)
                    nc.gpsimd.tensor_tensor(
                        out=pT[:, hf], in0=t[:, hf], in1=t[:, hf], op=MULT)
            else:
                pT = ppT_act.tile([128, QS], BF16_DT, tag="pT")
                nc.scalar.activation(
                    out=pT, in_=sT, func=mybir.ActivationFunctionType.Exp,
                    scale=float(EXPSCALE), bias=ln2_sb[:, 0:1])
            return pT

        def acc_slot(accs, h, qt):
            if qt < 7:
                return accs[h], 65 * qt
            return accs[2], 65 * h

        def pv(accs, kv, h, pT, init):
            for qt in range(NQT):
                acc, off = acc_slot(accs, h, qt)
                first_in_bank = qt == 0 or (qt == 7 and h == 0)
                nc.tensor.matmul(
                    acc[:, off:off + 65],
                    lhsT=pT[:, 128 * qt:128 * (qt + 1)],
                    rhs=v_sb[:, kv, 65 * h:65 * (h + 1)],
                    start=(init and kv == 0 and first_in_bank),
                    stop=(kv == NKV - 1),
                    skip_group_check=True,
                )

        def sv_reduce(accC):
            """sv[h] = sum of v over this head's Taylor kv tiles (+count)."""
            items = [(h, kv) for h in (0, 1) for kv in sorted(TK[h])]
            for i, (h, kv) in enumerate(items):
                nc.tensor.matmul(
                    accC[0:1, PSV0 + 65 * h:PSV0 + 65 * (h + 1)],
                    lhsT=ones_col[:, 0:1],
                    rhs=v_sb[:, kv, 65 * h:65 * (h + 1)],
                    start=False, stop=(i == len(items) - 1),
                    skip_group_check=True,
                )
            nc.vector.tensor_copy(out=sv_sb, in_=accC[0:1, PSV0:PSV0 + 130])

        def corrections(accs, h, init):
            """acc[q, :] += sv[h] for every q (K=1 broadcast matmuls)."""
            for qt in range(NQT):
                acc, off = acc_slot(accs, h, qt)
                first_in_bank = qt == 0 or (qt == 7 and h == 0)
                nc.tensor.matmul(
                    acc[:, off:off + 65],
                    lhsT=ones_row[0:1, 0:128],
                    rhs=sv_sb[0:1, 65 * h:65 * (h + 1)],
                    start=(init and first_in_bank), stop=True,
                    skip_group_check=True,
                )

        def norm_mul(accs, h, qts=tuple(range(NQT))):
            """Extract+normalize head h's accumulators into o_sb tiles.

            Only the (cheap) reciprocals and per-qt muls run at the
            superblock boundary; the transposes/copies into oT_sb are
            returned as deferred closures to spread over later iterations.
            """
            accH, accC = accs[h], accs[2]
            rec = pnorm.tile([128, 8], FP32, tag="rec")
            den7 = accH[:, 0:455].rearrange("p (s c) -> p s c", s=7)[:, :, 64]
            nc.vector.reciprocal(rec[:, 0:7], den7)
            nc.vector.reciprocal(rec[:, 7:8],
                                 accC[:, 65 * h + 64:65 * h + 65])
            outs = []
            for qt in qts:
                acc, off = acc_slot(accs, h, qt)
                o_sb = po.tile([128, 64], BF16_DT, tag="o_sb")
                nc.vector.tensor_scalar_mul(
                    out=o_sb, in0=acc[:, off:off + 64],
                    scalar1=rec[:, qt:qt + 1])
                outs.append((qt, o_sb))
            return outs

        def o_transpose(accC, qs, h, qt, o_sb, slot):
            """Transpose one normalized o tile into oT_sb (deferred)."""
            q0 = QS * qs
            trv = accC.bitcast(BF16_DT)
            trp = trv[:, TRB0 + 128 * slot:TRB0 + 128 * (slot + 1)]
            nc.tensor.transpose(trp[0:64, :], o_sb, ident)
            nc.vector.tensor_copy(
                out=oT_sb[0:64, h * N + q0 + 128 * qt:
                          h * N + q0 + 128 * (qt + 1)],
                in_=trp[0:64, :],
            )

        def norm_head(accs, qs, h, qts=tuple(range(NQT))):
            """Boundary norm with inline transposes (tail path)."""
            for i, (qt, o_sb) in enumerate(norm_mul(accs, h, qts)):
                o_transpose(accs[2], qs, h, qt, o_sb, i % 2)

        def outproj_piece(ch, ct, pool=None):
            if pool is None:
                ps = pbig.tile([128, CH], FP32, tag="big", name="ps")
            else:
                ps = pool.tile([128, CH], FP32, tag="sT", name="ps")
            for h in range(2):
                nc.tensor.matmul(
                    ps,
                    lhsT=wo_sb[0:DH, h * C + 128 * ct:h * C + 128 * (ct + 1)],
                    rhs=oT_sb[0:DH, h * N + CH * ch:h * N + CH * (ch + 1)],
                    start=(h == 0), stop=(h == 1),
                )
            st = so.tile([128, CH], BF16_DT, tag="st")
            nc.vector.tensor_scalar_add(
                out=st, in0=ps, scalar1=bo_sb[:, ct:ct + 1])
            nc.sync.dma_start(
                out=poutT[128 * ct:128 * (ct + 1), CH * ch:CH * (ch + 1)],
                in_=st,
            )

        # ---------- startup prefix ----------
        proj_qk(k8_sb, 128, 1, 0, pool=psT)
        proj_qk(q8_sb, 0, 0, 0, pool=psT)
        proj_qk(q8_sb, 0, 0, 1)
        proj_v(0, pool=psT)
        for kv in range(4):
            vtr(kv, pool=psT if kv % 2 else None)

        accs = [pacc.tile([128, 512], FP32, tag=t, name=t)
                for t in ("accA", "accB", "accC")]

        # Filler tasks drip-fed into the attention loop (1 per iteration).
        # The h0 block of qs0 has exactly 32 drains: 28 chunk tasks, q2, q3,
        # sv, and h0's late corrections (which must precede norm h0).
        filler = deque()
        for j in range(1, NCH):
            filler.append(lambda j=j: proj_qk(k8_sb, 128, 1, j))
            filler.append(lambda j=j: proj_v(j))
            filler.append(lambda j=j: (vtr(4 * j), vtr(4 * j + 1)))
            filler.append(lambda j=j: (vtr(4 * j + 2), vtr(4 * j + 3)))
        filler.append(lambda: proj_qk(q8_sb, 0, 0, 2))
        filler.append(lambda: proj_qk(q8_sb, 0, 0, 3))
        filler.append(lambda: sv_reduce(accs[2]))
        filler.append(lambda: corrections(accs, 0, init=False))
        # h1 block of qs0: h1's corrections INITIALIZE accB (h1's qs0 pvs
        # use start=False) and must land after norm-h0's transposes
        filler.append(lambda: proj_qk(q8_sb, 0, 0, 4))
        filler.append(lambda: corrections(accs, 1, init=True))
        for j in range(5, NCH):
            filler.append(lambda j=j: proj_qk(q8_sb, 0, 0, j))

        def drain_filler(nmax):
            for _ in range(min(nmax, len(filler))):
                filler.popleft()()

        # ---------- attention ----------
        # One head per iteration (h0 block, then h1 block, per superblock),
        # flat-pipelined with a ONE-ITERATION pv delay: at step i we emit
        # s_mm+pT producers for triple i+1 and consume (pv) triple i-1.
        # Every PE instruction's inputs are then a full iteration old, so no
        # engine ever stalls the PE stream (and through it, ACT/GPSIMD).
        triples = [(qs, h, kv)
                   for qs in range(NQS) for h in (0, 1) for kv in range(NKV)]

        def post_pv(qs, h, kv):
            """Events after the pv of (qs, h, kv) retires from the pipeline."""
            nonlocal accs
            if kv != NKV - 1:
                return
            if h == 0:
                # h1's first accC pv is one step behind (pv delay), so the
                # transpose hw-clear lands before h1's qt7 accumulation.
                # h1's corrections (accB init + accC slot) must come AFTER
                # these transposes — their start=True clears accC has_written.
                norm_head(accs, qs, 0)
                if qs > 0:
                    corrections(accs, 1, init=True)
                return
            if qs == NQS - 1:
                return  # tail handled after the loop
            norm_head(accs, qs, 1)
            accs = [pacc.tile([128, 512], FP32, tag=t, name=t)
                    for t in ("accA", "accB", "accC")]
            corrections(accs, 0, init=True)
            for ch in (2 * qs, 2 * qs + 1):
                for ct in range(4):
                    filler.append(lambda ch=ch, ct=ct: outproj_piece(ch, ct))

        pend = deque()
        pend.append((make_pT(s_mm(0, 0, 0), 0, 0), triples[0]))
        for i in range(len(triples)):
            if i + 1 < len(triples):
                nqs, nh, nkv = triples[i + 1]
                pend.append((make_pT(s_mm(nqs, nkv, nh), nkv, nh),
                             (nqs, nh, nkv)))
            drain_filler(1)
            if len(pend) > 1:
                pT, (qs, h, kv) = pend.popleft()
                pv(accs, kv, h, pT, qs == 0 and h == 0)
                post_pv(qs, h, kv)
        pT, (qs, h, kv) = pend.popleft()
        pv(accs, kv, h, pT, qs == 0 and h == 0)
        assert not pend

        # tail: outproj chunk needs only its own 4 qt of both heads
        # (h0 of qs3 was normed inline by post_pv)
        o1 = norm_mul(accs, 1)
        for i, (qt, o_sb) in enumerate(o1[0:4]):
            o_transpose(accs[2], qs, 1, qt, o_sb, i % 2)
        for ct in range(4):
            outproj_piece(2 * qs, ct, pool=psT if ct % 2 else None)
        for i, (qt, o_sb) in enumerate(o1[4:8]):
            o_transpose(accs[2], qs, 1, qt, o_sb, i % 2)
        for ct in range(4):
            outproj_piece(2 * qs + 1, ct, pool=psT if ct % 2 else None)
        assert not filler, len(filler)


_NC = None


def _build_nc():
    global _NC
    if _NC is None:
        nc = bacc.Bacc("TRN2", target_bir_lowering=False, debug=False,
                       num_devices=NCORES)
        with tile.TileContext(nc) as tc:
            _emit(tc)
        nc.finalize()
        _NC = nc
    return _NC


def _in_maps(x, w_qkv, b_qkv, w_out, b_out):
    x = np.asarray(x, dtype=np.float32)
    w_qkv = np.asarray(w_qkv, dtype=np.float32)
    b_qkv = np.asarray(b_qkv, dtype=np.float32)
    w_out = np.asarray(w_out, dtype=np.float32)
    b_out = np.asarray(b_out, dtype=np.float32)

    w4 = w_qkv.reshape(C, 3, H, DH)
    b4 = b_qkv.reshape(3, H, DH)
    xT_b = [np.ascontiguousarray(x[b].T).astype(BF16) for b in range(B)]

    maps = []
    for c in range(NCORES):
        b = c // 4
        h0, h1 = 2 * (c % 4), 2 * (c % 4) + 1
        # q/k blocks in [32,2]-split partition order:
        #   [h0 d0-31 | h1 d0-31 | h0 d32-63 | h1 d32-63]
        def qk_block(i):
            return np.concatenate(
                [w4[:, i, h0, 0:32], w4[:, i, h1, 0:32],
                 w4[:, i, h0, 32:64], w4[:, i, h1, 32:64]], axis=1)
        wl = np.concatenate(
            [qk_block(0), qk_block(1), w4[:, 2, h0], w4[:, 2, h1]],
            axis=1).astype(BF16)
        bq = np.zeros((3, 128), np.float32)
        for i, row in ((0, 0), (1, 1)):
            bq[row] = np.concatenate(
                [b4[i, h0, 0:32], b4[i, h1, 0:32],
                 b4[i, h0, 32:64], b4[i, h1, 32:64]])
        bq[2] = np.concatenate([b4[2, h0], b4[2, h1]])
        wo = np.concatenate(
            [w_out[DH * h0:DH * (h0 + 1)], w_out[DH * h1:DH * (h1 + 1)]],
            axis=1).astype(BF16)
        bo = (b_out.reshape(4, 128) if c % 4 == 0
              else np.zeros((4, 128), np.float32))
        maps.append({
            "xT": xT_b[b],
            "wqkv": np.ascontiguousarray(wl),
            "bqkv": bq,
            "wout": np.ascontiguousarray(wo),
            "bout": np.ascontiguousarray(bo.astype(np.float32)),
        })
    return maps


def kernel(x, w_qkv, b_qkv, w_out, b_out, _trace=False, **_trace_kwargs):
    nc = _build_nc()
    maps = _in_maps(x, w_qkv, b_qkv, w_out, b_out)
    res = run_bass_kernel_spmd(nc, maps, core_ids=list(range(NCORES)),
                               trace=_trace, **_trace_kwargs)
    parts = [np.asarray(r["poutT"]).astype(np.float32) for r in res.results]
    out = np.empty((B, N, C), dtype=np.float32)
    for b in range(B):
        acc = parts[4 * b]
        for i in range(1, 4):
            acc = acc + parts[4 * b + i]
        out[b] = acc.T
    if _trace:
        return out, res
    return out


# revision 34
# speedup vs baseline: 1.0409x; 1.0318x over previous
"""Multi-head attention kernel for Trainium2, SPMD over 8 NeuronCores.

Problem: B=2, N=4096, C=512, H=8 heads, DH=64. fp32 I/O.
Sharding: core c -> batch b=c//4, heads {2*(c%4), 2*(c%4)+1}.
Each core computes its 2 heads' attention + a partial output projection
(transposed layout [C, N], bf16); the host sums the 4 partials per batch
and transposes back.

v2 design (ACT was the bottleneck at 267us busy / 327us total):
- S^T matmuls run in fp8e4 DoubleRow (0.5 cyc/row): q,k are projected,
  scaled by 16 and quantized to fp8 in a [32, 2, N] layout (contraction
  DH=64 split as Ki=32 partitions x Ko=2), halving the dominant PE term.
- ~31% of the exp tiles are offloaded from ACT: softmax weights for
  those (kv, head) tiles use the 2nd-order Taylor form
  (1+S)^2 + 1 = 2*exp(S) + O(S^3)  (S has std ~0.07, |S|<0.65)
  computed as one DVE tensor_scalar (t = 1 + S, reading PSUM) plus one
  GPSIMD tensor_tensor (t*t). ACT tiles compute 2*exp(S) via bias=ln2,
  and the "+1" constant is folded in algebraically: a per-head
  sum-of-v over the Taylor kv tiles (sv) is accumulated into the o/den
  accumulators with K=1 broadcast matmuls, so normalization divides the
  consistent 2x-scaled numerator/denominator.
- next-iteration pT producers are emitted before the boundary norm work
  so ACT/DVE/GPSIMD never wait on the norm chain; norm uses batched
  reciprocals and ping-pong transpose slots carved out of accC's free
  PSUM space.
- startup: PE-warmup transpose chain (p-state ramp), x chunk0 + the
  q/k halves of w are DMA'd first across 3 queues.
"""

import math
from collections import deque

import numpy as np
import ml_dtypes

import concourse.tile as tile
from concourse import bacc, mybir
from concourse.bass_utils import run_bass_kernel_spmd
from concourse.masks import make_identity

BF16 = ml_dtypes.bfloat16
E4M3 = ml_dtypes.float8_e4m3fn

B, N, C, H = 2, 4096, 512, 8
DH = C // H          # 64
NCORES = 8
SCALE = C ** -0.5    # reference scales by hidden_dim, not head_dim
PRE = 16.0           # q/k prescale into fp8e4 range
EXPSCALE = SCALE / (PRE * PRE)
LN2 = math.log(2.0)

QS = 1024            # query superblock
NQS = N // QS        # 4
NKV = N // 128       # 32 kv tiles
NQT = QS // 128      # 8 query tiles per superblock
CH = 512             # token chunk for projections
NCH = N // CH        # 8

FP32 = mybir.dt.float32
BF16_DT = mybir.dt.bfloat16
FP8 = mybir.dt.float8e4

# Taylor (DVE+GPSIMD) kv tiles per head; same sets for every superblock.
TK = (frozenset(range(0, 30, 3)), frozenset(range(1, 31, 3)))  # 10 + 10

PSV0 = 136     # fp32 col in accC where the sv accumulator lives [136:266)
TRB0 = 544     # bf16 col in accC for transpose ping-pong slots (2x128)
WARMUP = 40    # PE p-state warmup transposes

ADD = mybir.AluOpType.add
MULT = mybir.AluOpType.mult


def _emit(tc):
    nc = tc.nc
    xT = nc.dram_tensor("xT", [C, N], BF16_DT, kind="ExternalInput").ap()
    wqkv = nc.dram_tensor("wqkv", [C, 6 * DH], BF16_DT, kind="ExternalInput").ap()
    bqkv = nc.dram_tensor("bqkv", [3, 128], FP32, kind="ExternalInput").ap()
    wout = nc.dram_tensor("wout", [DH, 2 * C], BF16_DT, kind="ExternalInput").ap()
    bout = nc.dram_tensor("bout", [4, 128], FP32, kind="ExternalInput").ap()
    poutT = nc.dram_tensor("poutT", [C, N], BF16_DT, kind="ExternalOutput").ap()

    with (
        tc.tile_pool(name="singles", bufs=1) as singles,
        tc.tile_pool(name="psum_big", bufs=1, space="PSUM") as pbig,
        tc.tile_pool(name="psum_sT", bufs=2, space="PSUM") as psT,
        tc.tile_pool(name="psum_acc", bufs=1, space="PSUM") as pacc,
        # separate pT pools per producer engine: a shared pool would create
        # WAW buffer deps chaining ACT behind the GPSIMD tt stream
        tc.tile_pool(name="pT_act", bufs=5) as ppT_act,
        tc.tile_pool(name="pT_tay", bufs=4) as ppT_tay,
        tc.tile_pool(name="tay_pool", bufs=6) as ptay,
        tc.tile_pool(name="q8_pool", bufs=3) as pq8,
        tc.tile_pool(name="norm_pool", bufs=4) as pnorm,
        tc.tile_pool(name="o_pool", bufs=18) as po,
        tc.tile_pool(name="stage_out", bufs=4) as so,
    ):
        # --- resident SBUF tensors ---
        xT_sb = singles.tile([128, 4, N], BF16_DT)
        w_sb = singles.tile([128, 4, 6 * DH], BF16_DT)
        bq_sb = singles.tile([128, 3], FP32)
        wo_sb = singles.tile([128, 2 * C], BF16_DT)
        bo_sb = singles.tile([128, 4], FP32)
        ident = singles.tile([128, 128], BF16_DT)
        ones_col = singles.tile([128, 1], BF16_DT)
        ones_row = singles.tile([1, 128], FP32)
        # q/k fp8 [32,2]-split: parts 0-31 h0, 32-63 h1; free (j, h... )
        # element (p, j, n) = PRE * q_{d = 32j + p}(n), head = p//32
        q8_sb = singles.tile([64, 2, N], FP8)
        k8_sb = singles.tile([64, 2, N], FP8)
        vT_sb = singles.tile([128, N], BF16_DT)
        v_sb = singles.tile([128, NKV, 130], BF16_DT)
        oT_sb = singles.tile([64, 2 * N], BF16_DT)
        sv_sb = singles.tile([1, 130], FP32)
        ln2_sb = singles.tile([128, 1], FP32)
        warm = singles.tile([128, 1], FP32)

        make_identity(nc, ident)
        nc.vector.memset(ones_col, 1.0)
        nc.vector.memset(ones_row, 1.0)
        nc.vector.memset(ln2_sb, float(LN2))
        nc.vector.memset(v_sb[:, :, 64:65], 1.0)
        nc.vector.memset(v_sb[:, :, 129:130], 1.0)
        nc.vector.memset(warm, 0.0)
        # load the Exp table set during setup
        nc.scalar.activation(out=warm, in_=warm,
                             func=mybir.ActivationFunctionType.Exp)
        # PE p-state warmup: keep PE continuously busy through the DMA wait
        for _ in range(WARMUP):
            wt = pbig.tile([128, 128], BF16_DT, tag="big", name="warm")
            nc.tensor.transpose(wt, ident, ident)

        # --- DMA schedule: x ch0 + w(qk) first, on 3 HWDGE queues ---
        QQ = (nc.sync, nc.scalar, nc.gpsimd)

        def x_piece(ch, kt):
            # sync/scalar HWDGE only: gpsimd's DMA issue occupies the Pool
            # engine, which the Taylor tt stream needs
            QQ[(4 * ch + kt) % 2].dma_start(
                out=xT_sb[:, kt, CH * ch:CH * (ch + 1)],
                in_=xT[128 * kt:128 * (kt + 1), CH * ch:CH * (ch + 1)])

        for kt in range(4):
            x_piece(0, kt)
        for kt in range(4):
            (nc.sync if kt % 2 else nc.scalar).dma_start(
                out=w_sb[:, kt, 0:256], in_=wqkv[128 * kt:128 * (kt + 1), 0:256])
        for j in range(3):
            nc.scalar.dma_start(out=bq_sb[:, j:j + 1], in_=bqkv[j, :])
        for kt in range(4):
            x_piece(1, kt)
        for kt in range(4):
            (nc.sync if kt % 2 else nc.scalar).dma_start(
                out=w_sb[:, kt, 256:384],
                in_=wqkv[128 * kt:128 * (kt + 1), 256:384])
        nc.sync.dma_start(out=wo_sb[0:DH, :], in_=wout[:, :])
        for j in range(4):
            nc.scalar.dma_start(out=bo_sb[:, j:j + 1], in_=bout[j, :])
        for ch in range(2, NCH):
            for kt in range(4):
                x_piece(ch, kt)

        # ---------- emission helpers ----------

        def proj_qk(dst8, wcol0, bcol, ch, pool=None):
            """Project one 512-token chunk of q or k into fp8 [32,2] layout."""
            sl = slice(CH * ch, CH * (ch + 1))
            if pool is None:
                ps = pbig.tile([128, CH], FP32, tag="big", name="ps")
            else:
                ps = pool.tile([128, CH], FP32, tag="sT", name="ps")
            for kt in range(4):
                nc.tensor.matmul(
                    ps, lhsT=w_sb[:, kt, wcol0:wcol0 + 128],
                    rhs=xT_sb[:, kt, sl], start=(kt == 0), stop=(kt == 3))
            # rows 0-63 = j0 (both heads): convert in place
            nc.vector.tensor_scalar(
                out=dst8[0:64, 0, sl], in0=ps[0:64, :],
                scalar1=bq_sb[0:64, bcol:bcol + 1], scalar2=float(PRE),
                op0=ADD, op1=MULT)
            # rows 64-127 = j1: convert, then partition-shift via DMA
            qt8 = pq8.tile([128, CH], FP8, tag="q8")
            nc.vector.tensor_scalar(
                out=qt8[64:128, :], in0=ps[64:128, :],
                scalar1=bq_sb[64:128, bcol:bcol + 1], scalar2=float(PRE),
                op0=ADD, op1=MULT)
            # sync queue, NOT scalar: a DMA waiting on its ts input would
            # stall every exp behind it in the ACT queue
            nc.sync.dma_start(out=dst8[0:64, 1, sl], in_=qt8[64:128, :])

        def proj_v(ch, pool=None):
            sl = slice(CH * ch, CH * (ch + 1))
            if pool is None:
                ps = pbig.tile([128, CH], FP32, tag="big", name="ps")
            else:
                ps = pool.tile([128, CH], FP32, tag="sT", name="ps")
            for kt in range(4):
                nc.tensor.matmul(
                    ps, lhsT=w_sb[:, kt, 256:384],
                    rhs=xT_sb[:, kt, sl], start=(kt == 0), stop=(kt == 3))
            nc.vector.tensor_scalar_add(
                out=vT_sb[:, sl], in0=ps, scalar1=bq_sb[:, 2:3])

        def vtr(kv, pool=None):
            """Transpose v^T tile kv into v_sb [tok, d] layout."""
            if pool is None:
                trp = pbig.tile([128, 128], BF16_DT, tag="big", name="trp")
            else:
                trp = pool.tile([128, 128], BF16_DT, tag="sT", name="trp")
            nc.tensor.transpose(trp, vT_sb[:, 128 * kv:128 * (kv + 1)], ident)
            src = trp.rearrange("p (j c) -> p j c", j=2)
            dst = v_sb[:, kv, 0:130].rearrange("p (j c) -> p j c", j=2)
            nc.vector.tensor_copy(out=dst[:, :, 0:64], in_=src)

        def vtr4(kv0):
            """Transpose 4 v^T tiles via ONE pbig tile + ONE strided copy.

            The 4 transposes share one allocation (no intra-task WAW
            round-trips through DVE stalling the in-order PE stream).
            """
            trp = pbig.tile([128, 4, 128], BF16_DT, tag="big", name="trp4")
            for i in range(4):
                nc.tensor.transpose(
                    trp[:, i, :], vT_sb[:, 128 * (kv0 + i):128 * (kv0 + i + 1)],
                    ident)
            src = trp.rearrange("p k (j c) -> p k j c", j=2)
            dst = v_sb[:, kv0:kv0 + 4, 0:130].rearrange(
                "p k (j c) -> p k j c", j=2)
            nc.vector.tensor_copy(out=dst[:, :, :, 0:64], in_=src)

        def s_mm(qs, kv, h):
            """S^T = k_tile^T q_super via fp8e4 DoubleRow."""
            q0 = QS * qs
            sT = psT.tile([128, QS], FP32, tag="sT")
            for half in range(2):
                nc.tensor.matmul(
                    sT[:, 512 * half:512 * (half + 1)],
                    lhsT=k8_sb[32 * h:32 * (h + 1), :, 128 * kv:128 * (kv + 1)],
                    rhs=q8_sb[32 * h:32 * (h + 1), :,
                              q0 + 512 * half:q0 + 512 * (half + 1)],
                    start=True, stop=True,
                    perf_mode=mybir.MatmulPerfMode.DoubleRow,
                )
            return sT

        def make_pT(sT, kv, h):
            """p tile: ACT 2*exp(S), or DVE+GPSIMD (1+S)^2 (Taylor tiles).

            Taylor tiles are processed in 512-wide halves so the first pv
            matmuls can start after ~half the ts+tt latency, and the psum
            buf is released by the (cheap) DVE ts rather than the tt.
            """
            if kv in TK[h]:
                pT = ppT_tay.tile([128, QS], BF16_DT, tag="pT")
                t = ptay.tile([128, QS], BF16_DT, tag="tay")
                for hf in (slice(0, 512), slice(512, 1024)):
                    nc.vector.tensor_scalar(
                        out=t[:, hf], in0=sT[:, hf], scalar1=float(EXPSCALE),
                        scalar2=1.0, op0=MULT, op1=ADD)
                    nc.gpsimd.tensor_tensor(
                        out=pT[:, hf], in0=t[:, hf], in1=t[:, hf], op=MULT)
            else:
                pT = ppT_act.tile([128, QS], BF16_DT, tag="pT")
                nc.scalar.activation(
                    out=pT, in_=sT, func=mybir.ActivationFunctionType.Exp,
                    scale=float(EXPSCALE), bias=ln2_sb[:, 0:1])
            return pT

        def acc_slot(accs, h, qt):
            if qt < 7:
                return accs[h], 65 * qt
            return accs[2], 65 * h

        def pv(accs, kv, h, pT, init):
            for qt in range(NQT):
                acc, off = acc_slot(accs, h, qt)
                first_in_bank = qt == 0 or (qt == 7 and h == 0)
                nc.tensor.matmul(
                    acc[:, off:off + 65],
                    lhsT=pT[:, 128 * qt:128 * (qt + 1)],
                    rhs=v_sb[:, kv, 65 * h:65 * (h + 1)],
                    start=(init and kv == 0 and first_in_bank),
                    stop=(kv == NKV - 1),
                    skip_group_check=True,
                )

        def sv_reduce(accC):
            """sv[h] = sum of v over this head's Taylor kv tiles (+count)."""
            items = [(h, kv) for h in (0, 1) for kv in sorted(TK[h])]
            for i, (h, kv) in enumerate(items):
                nc.tensor.matmul(
                    accC[0:1, PSV0 + 65 * h:PSV0 + 65 * (h + 1)],
                    lhsT=ones_col[:, 0:1],
                    rhs=v_sb[:, kv, 65 * h:65 * (h + 1)],
                    start=False, stop=(i == len(items) - 1),
                    skip_group_check=True,
                )
            nc.vector.tensor_copy(out=sv_sb, in_=accC[0:1, PSV0:PSV0 + 130])

        def corrections(accs, h, init):
            """acc[q, :] += sv[h] for every q (K=1 broadcast matmuls)."""
            for qt in range(NQT):
                acc, off = acc_slot(accs, h, qt)
                first_in_bank = qt == 0 or (qt == 7 and h == 0)
                nc.tensor.matmul(
                    acc[:, off:off + 65],
                    lhsT=ones_row[0:1, 0:128],
                    rhs=sv_sb[0:1, 65 * h:65 * (h + 1)],
                    start=(init and first_in_bank), stop=True,
                    skip_group_check=True,
                )

        def norm_mul(accs, h, qts=tuple(range(NQT))):
            """Extract+normalize head h's accumulators into o_sb tiles.

            Only the (cheap) reciprocals and per-qt muls run at the
            superblock boundary; the transposes/copies into oT_sb are
            returned as deferred closures to spread over later iterations.
            """
            accH, accC = accs[h], accs[2]
            rec = pnorm.tile([128, 8], FP32, tag="rec")
            den7 = accH[:, 0:455].rearrange("p (s c) -> p s c", s=7)[:, :, 64]
            nc.vector.reciprocal(rec[:, 0:7], den7)
            nc.vector.reciprocal(rec[:, 7:8],
                                 accC[:, 65 * h + 64:65 * h + 65])
            outs = []
            for qt in qts:
                acc, off = acc_slot(accs, h, qt)
                o_sb = po.tile([128, 64], BF16_DT, tag="o_sb")
                nc.vector.tensor_scalar_mul(
                    out=o_sb, in0=acc[:, off:off + 64],
                    scalar1=rec[:, qt:qt + 1])
                outs.append((qt, o_sb))
            return outs

        def o_transpose(accC, qs, h, qt, o_sb, slot):
            """Transpose one normalized o tile into oT_sb (deferred)."""
            q0 = QS * qs
            trv = accC.bitcast(BF16_DT)
            trp = trv[:, TRB0 + 128 * slot:TRB0 + 128 * (slot + 1)]
            nc.tensor.transpose(trp[0:64, :], o_sb, ident)
            nc.vector.tensor_copy(
                out=oT_sb[0:64, h * N + q0 + 128 * qt:
                          h * N + q0 + 128 * (qt + 1)],
                in_=trp[0:64, :],
            )

        def norm_head(accs, qs, h, qts=tuple(range(NQT))):
            """Boundary norm with inline transposes (tail path)."""
            for i, (qt, o_sb) in enumerate(norm_mul(accs, h, qts)):
                o_transpose(accs[2], qs, h, qt, o_sb, i % 2)

        def outproj_piece(ch, ct, pool=None):
            if pool is None:
                ps = pbig.tile([128, CH], FP32, tag="big", name="ps")
            else:
                ps = pool.tile([128, CH], FP32, tag="sT", name="ps")
            for h in range(2):
                nc.tensor.matmul(
                    ps,
                    lhsT=wo_sb[0:DH, h * C + 128 * ct:h * C + 128 * (ct + 1)],
                    rhs=oT_sb[0:DH, h * N + CH * ch:h * N + CH * (ch + 1)],
                    start=(h == 0), stop=(h == 1),
                )
            st = so.tile([128, CH], BF16_DT, tag="st")
            nc.vector.tensor_scalar_add(
                out=st, in0=ps, scalar1=bo_sb[:, ct:ct + 1])
            nc.sync.dma_start(
                out=poutT[128 * ct:128 * (ct + 1), CH * ch:CH * (ch + 1)],
                in_=st,
            )

        # ---------- startup prefix ----------
        proj_qk(k8_sb, 128, 1, 0, pool=psT)
        proj_qk(q8_sb, 0, 0, 0, pool=psT)
        proj_qk(q8_sb, 0, 0, 1)
        proj_v(0, pool=psT)
        for kv in range(4):
            vtr(kv, pool=psT if kv % 2 else None)

        accs = [pacc.tile([128, 512], FP32, tag=t, name=t)
                for t in ("accA", "accB", "accC")]

        # Filler tasks drip-fed into the attention loop (1 per iteration).
        # The h0 block of qs0 has exactly 32 drains: 28 chunk tasks, q2, q3,
        # sv, and h0's late corrections (which must precede norm h0).
        filler = deque()
        for j in range(1, NCH):
            filler.append(lambda j=j: proj_qk(k8_sb, 128, 1, j))
            filler.append(lambda j=j: proj_v(j))
            filler.append(lambda j=j: vtr4(4 * j))
        filler.append(lambda: proj_qk(q8_sb, 0, 0, 2))
        filler.append(lambda: proj_qk(q8_sb, 0, 0, 3))
        filler.append(lambda: sv_reduce(accs[2]))
        filler.append(lambda: corrections(accs, 0, init=False))
        for j in range(4, NCH):
            filler.append(lambda j=j: proj_qk(q8_sb, 0, 0, j))
        # h1's corrections INITIALIZE accB (h1's qs0 pvs use start=False);
        # they must drain at step >= 34, after norm-h0's transposes at step
        # 33 (their start=True clears accC has_written). Pad to position 33.
        while len(filler) < 33:
            filler.append(lambda: None)
        filler.append(lambda: corrections(accs, 1, init=True))

        def drain_filler(nmax):
            for _ in range(min(nmax, len(filler))):
                filler.popleft()()

        # ---------- attention ----------
        # One head per iteration (h0 block, then h1 block, per superblock),
        # flat-pipelined with a ONE-ITERATION pv delay: at step i we emit
        # s_mm+pT producers for triple i+1 and consume (pv) triple i-1.
        # Every PE instruction's inputs are then a full iteration old, so no
        # engine ever stalls the PE stream (and through it, ACT/GPSIMD).
        triples = [(qs, h, kv)
                   for qs in range(NQS) for h in (0, 1) for kv in range(NKV)]

        def post_pv(qs, h, kv):
            """Events after the pv of (qs, h, kv) retires from the pipeline."""
            nonlocal accs
            if kv != NKV - 1:
                return
            if h == 0:
                # h1's first accC pv is one step behind (pv delay), so the
                # transpose hw-clear lands before h1's qt7 accumulation.
                # h1's corrections (accB init + accC slot) must come AFTER
                # these transposes — their start=True clears accC has_written.
                norm_head(accs, qs, 0)
                if qs > 0:
                    corrections(accs, 1, init=True)
                return
            if qs == NQS - 1:
                return  # tail handled after the loop
            norm_head(accs, qs, 1)
            accs = [pacc.tile([128, 512], FP32, tag=t, name=t)
                    for t in ("accA", "accB", "accC")]
            corrections(accs, 0, init=True)
            for ch in (2 * qs, 2 * qs + 1):
                for ct in range(4):
                    filler.append(lambda ch=ch, ct=ct: outproj_piece(ch, ct))

        pend = deque()
        pend.append((make_pT(s_mm(0, 0, 0), 0, 0), triples[0]))
        for i in range(len(triples)):
            if i + 1 < len(triples):
                nqs, nh, nkv = triples[i + 1]
                pend.append((make_pT(s_mm(nqs, nkv, nh), nkv, nh),
                             (nqs, nh, nkv)))
            drain_filler(1)
            if len(pend) > 2:
                pT, (qs, h, kv) = pend.popleft()
                pv(accs, kv, h, pT, qs == 0 and h == 0)
                post_pv(qs, h, kv)
        while pend:
            pT, (qs, h, kv) = pend.popleft()
            pv(accs, kv, h, pT, qs == 0 and h == 0)
            post_pv(qs, h, kv)

        # tail: outproj chunk needs only its own 4 qt of both heads
        # (h0 of qs3 was normed inline by post_pv)
        o1 = norm_mul(accs, 1)
        for i, (qt, o_sb) in enumerate(o1[0:4]):
            o_transpose(accs[2], qs, 1, qt, o_sb, i % 2)
        for ct in range(4):
            outproj_piece(2 * qs, ct, pool=psT if ct % 2 else None)
        for i, (qt, o_sb) in enumerate(o1[4:8]):
            o_transpose(accs[2], qs, 1, qt, o_sb, i % 2)
        for ct in range(4):
            outproj_piece(2 * qs + 1, ct, pool=psT if ct % 2 else None)
        assert not filler, len(filler)


_NC = None


def _build_nc():
    global _NC
    if _NC is None:
        nc = bacc.Bacc("TRN2", target_bir_lowering=False, debug=False,
                       num_devices=NCORES)
        with tile.TileContext(nc) as tc:
            _emit(tc)
        nc.finalize()
        _NC = nc
    return _NC


def _in_maps(x, w_qkv, b_qkv, w_out, b_out):
    x = np.asarray(x, dtype=np.float32)
    w_qkv = np.asarray(w_qkv, dtype=np.float32)
    b_qkv = np.asarray(b_qkv, dtype=np.float32)
    w_out = np.asarray(w_out, dtype=np.float32)
    b_out = np.asarray(b_out, dtype=np.float32)

    w4 = w_qkv.reshape(C, 3, H, DH)
    b4 = b_qkv.reshape(3, H, DH)
    xT_b = [np.ascontiguousarray(x[b].T).astype(BF16) for b in range(B)]

    maps = []
    for c in range(NCORES):
        b = c // 4
        h0, h1 = 2 * (c % 4), 2 * (c % 4) + 1
        # q/k blocks in [32,2]-split partition order:
        #   [h0 d0-31 | h1 d0-31 | h0 d32-63 | h1 d32-63]
        def qk_block(i):
            return np.concatenate(
                [w4[:, i, h0, 0:32], w4[:, i, h1, 0:32],
                 w4[:, i, h0, 32:64], w4[:, i, h1, 32:64]], axis=1)
        wl = np.concatenate(
            [qk_block(0), qk_block(1), w4[:, 2, h0], w4[:, 2, h1]],
            axis=1).astype(BF16)
        bq = np.zeros((3, 128), np.float32)
        for i, row in ((0, 0), (1, 1)):
            bq[row] = np.concatenate(
                [b4[i, h0, 0:32], b4[i, h1, 0:32],
                 b4[i, h0, 32:64], b4[i, h1, 32:64]])
        bq[2] = np.concatenate([b4[2, h0], b4[2, h1]])
        wo = np.concatenate(
            [w_out[DH * h0:DH * (h0 + 1)], w_out[DH * h1:DH * (h1 + 1)]],
            axis=1).astype(BF16)
        bo = (b_out.reshape(4, 128) if c % 4 == 0
              else np.zeros((4, 128), np.float32))
        maps.append({
            "xT": xT_b[b],
            "wqkv": np.ascontiguousarray(wl),
            "bqkv": bq,
            "wout": np.ascontiguousarray(wo),
            "bout": np.ascontiguousarray(bo.astype(np.float32)),
        })
    return maps


def kernel(x, w_qkv, b_qkv, w_out, b_out, _trace=False, **_trace_kwargs):
    nc = _build_nc()
    maps = _in_maps(x, w_qkv, b_qkv, w_out, b_out)
    res = run_bass_kernel_spmd(nc, maps, core_ids=list(range(NCORES)),
                               trace=_trace, **_trace_kwargs)
    parts = [np.asarray(r["poutT"]).astype(np.float32) for r in res.results]
    out = np.empty((B, N, C), dtype=np.float32)
    for b in range(B):
        acc = parts[4 * b]
        for i in range(1, 4):
            acc = acc + parts[4 * b + i]
        out[b] = acc.T
    if _trace:
        return out, res
    return out


# revision 35
# speedup vs baseline: 1.2700x; 1.2201x over previous
"""Multi-head attention kernel for Trainium2, SPMD over 8 NeuronCores.

Problem: B=2, N=4096, C=512, H=8 heads, DH=64. fp32 I/O.
Sharding: core c -> batch b=c//4, heads {2*(c%4), 2*(c%4)+1}.
Each core computes its 2 heads' attention + a partial output projection
(transposed layout [C, N]); the host sums the 4 partials per batch and
transposes back.

The scalar engine (exp) is the bottleneck (~33.5M exps/core), so emission
is organized to keep it saturated:
- minimal projection prefix (k/v/q for the first tiles), then the
  flash-attention loop starts; remaining projection work is drip-fed as
  "filler" tasks into the loop's PE slack
- one shared single-buffer PSUM bank ("big") serves projections,
  transposes and the output projection so all pools fit in 8 banks
- at superblock boundaries the next block's first S^T/exp pair is peeled
  ahead of the normalization pass
"""

from collections import deque

import numpy as np
import ml_dtypes

import concourse.tile as tile
from concourse import bacc, mybir
from concourse.bass_utils import run_bass_kernel_spmd
from concourse.masks import make_identity

BF16 = ml_dtypes.bfloat16

B, N, C, H = 2, 4096, 512, 8
DH = C // H          # 64
NCORES = 8
SCALE = C ** -0.5    # reference scales by hidden_dim, not head_dim

QS = 1024            # query superblock (exp free dim)
NQS = N // QS        # 4
NKV = N // 128       # 32 kv tiles
NQT = QS // 128      # 8 query tiles per superblock
CH = 512             # token chunk for projections
NCH = N // CH        # 8

FP32 = mybir.dt.float32
BF16_DT = mybir.dt.bfloat16

DEBUG_DUMPS = False


def _emit(tc):
    nc = tc.nc
    xT = nc.dram_tensor("xT", [C, N], BF16_DT, kind="ExternalInput").ap()
    wqkv = nc.dram_tensor("wqkv", [C, 6 * DH], BF16_DT, kind="ExternalInput").ap()
    bqkv = nc.dram_tensor("bqkv", [5, 128], FP32, kind="ExternalInput").ap()
    wout = nc.dram_tensor("wout", [DH, 2 * C], BF16_DT, kind="ExternalInput").ap()
    bout = nc.dram_tensor("bout", [4, 128], FP32, kind="ExternalInput").ap()
    poutT = nc.dram_tensor("poutT", [C, N], FP32, kind="ExternalOutput").ap()

    with (
        tc.tile_pool(name="singles", bufs=1) as singles,
        tc.tile_pool(name="psum_big", bufs=1, space="PSUM") as pbig,
        tc.tile_pool(name="psum_sT", bufs=2, space="PSUM") as psT,
        tc.tile_pool(name="psum_acc", bufs=1, space="PSUM") as pacc,
        tc.tile_pool(name="pT_pool", bufs=6) as ppT,
        tc.tile_pool(name="qtmp_pool", bufs=3) as pqtmp,
        tc.tile_pool(name="norm_pool", bufs=4) as pnorm,
        tc.tile_pool(name="stage_out", bufs=4) as so,
    ):
        # --- resident SBUF tensors ---
        xT_sb = singles.tile([128, 4, N], BF16_DT)     # x^T, 4 k-tiles
        w_sb = singles.tile([128, 4, 6 * DH], BF16_DT)  # w_qkv local, 4 k-tiles
        bq_sb = singles.tile([128, 5], FP32)
        wo_sb = singles.tile([128, 2 * C], BF16_DT)    # [64 used, h0 cols | h1 cols]
        bo_sb = singles.tile([128, 4], FP32)
        ident = singles.tile([128, 128], BF16_DT)
        # q/k in [d, tok] layout, both heads on partitions 0-63:
        #   cols 0..N-1 = head0, cols N..2N-1 = head1
        q_sb = singles.tile([128, 2 * N], BF16_DT)
        k_sb = singles.tile([128, 2 * N], BF16_DT)
        vT_sb = singles.tile([128, N], BF16_DT)        # v^T [d(2 heads), tok]
        # v in [tok, d] layout per kv tile: [v_h0(64) | 1 | v_h1(64) | 1]
        v_sb = singles.tile([128, NKV, 130], BF16_DT)
        # normalized attention output, transposed: [d, tok];
        # parts 0-63, cols 0..N-1 = h0, N..2N-1 = h1
        oT_sb = singles.tile([128, 2 * N], BF16_DT)
        warm = singles.tile([128, 1], FP32)

        # xT loaded per (token-chunk, k-tile) so the first projections can
        # start after ~1MB instead of the full 4MB
        for kt in range(4):
            nc.sync.dma_start(out=w_sb[:, kt, :], in_=wqkv[128 * kt:128 * (kt + 1), :])
        for ch in range(NCH):
            for kt in range(4):
                eng = nc.sync if kt % 2 == 0 else nc.gpsimd
                eng.dma_start(
                    out=xT_sb[:, kt, CH * ch:CH * (ch + 1)],
                    in_=xT[128 * kt:128 * (kt + 1), CH * ch:CH * (ch + 1)])
        for j in range(5):
            nc.sync.dma_start(out=bq_sb[:, j:j + 1], in_=bqkv[j, :])
        nc.sync.dma_start(out=wo_sb[0:DH, :], in_=wout[:, :])
        for j in range(4):
            nc.sync.dma_start(out=bo_sb[:, j:j + 1], in_=bout[j, :])
        make_identity(nc, ident)
        nc.vector.memset(v_sb[:, :, 64:65], 1.0)
        nc.vector.memset(v_sb[:, :, 129:130], 1.0)
        # dummy exp so the ACT Exp table set loads during the setup phase
        nc.vector.memset(warm, 0.0)
        nc.scalar.activation(out=warm, in_=warm,
                             func=mybir.ActivationFunctionType.Exp)

        # ---------- emission helpers ----------

        def proj(dst, wcol0, ch, pool=None):
            """Project one 512-token chunk for q/k/v (M=128, both heads).

            dst is q_sb/k_sb (head-split layout, via DMA partition shift for
            head1) or vT_sb (kept packed). `pool` lets the pre-attention
            prefix borrow the idle sT psum slots for extra overlap.
            """
            sl = slice(CH * ch, CH * (ch + 1))
            if pool is None:
                ps = pbig.tile([128, CH], FP32, tag="big", name="ps")
            else:
                ps = pool.tile([128, CH], FP32, tag="sT", name="ps")
            for kt in range(4):
                nc.tensor.matmul(
                    ps,
                    lhsT=w_sb[:, kt, wcol0:wcol0 + 2 * DH],
                    rhs=xT_sb[:, kt, sl],
                    start=(kt == 0), stop=(kt == 3),
                )
            bias_col = wcol0 // (2 * DH)
            if dst is vT_sb:
                nc.vector.tensor_scalar_add(
                    out=vT_sb[:, sl], in0=ps, scalar1=bq_sb[:, 4:5])
                return
            # q/k bias columns: q -> [0|1], k -> [2|3] stacked as [128,1]
            bcol = 0 if wcol0 == 0 else 2
            qt_ = pqtmp.tile([128, CH], BF16_DT, tag="qtmp")
            nc.vector.tensor_scalar_add(
                out=qt_[0:DH, :], in0=ps[0:DH, :],
                scalar1=bq_sb[0:DH, bcol:bcol + 1])
            nc.vector.tensor_scalar_add(
                out=qt_[DH:128, :], in0=ps[DH:128, :],
                scalar1=bq_sb[DH:128, bcol + 1:bcol + 2])
            nc.vector.tensor_copy(out=dst[0:DH, sl], in_=qt_[0:DH, :])
            # head1 rows 64-127 -> partitions 0-63 at col offset N (DMA
            # shift). Scalar engine's HWDGE queue: empty, so these never
            # wait behind the bulk xT loads on the sync queue.
            nc.scalar.dma_start(out=dst[0:DH, N + CH * ch:N + CH * (ch + 1)],
                                in_=qt_[DH:128, :])

        def vtr(kv, pool=None):
            """Transpose v^T tile kv into v_sb [tok, d] layout."""
            if pool is None:
                trp = pbig.tile([128, 128], BF16_DT, tag="big", name="trp")
            else:
                trp = pool.tile([128, 128], BF16_DT, tag="sT", name="trp")
            nc.tensor.transpose(trp, vT_sb[:, 128 * kv:128 * (kv + 1)], ident)
            nc.vector.tensor_copy(out=v_sb[:, kv, 0:64], in_=trp[:, 0:64])
            nc.vector.tensor_copy(out=v_sb[:, kv, 65:129], in_=trp[:, 64:128])

        def s_mm(qs, kv, h):
            """S^T = k_tile^T q_super (PE part only)."""
            q0 = QS * qs
            sT = psT.tile([128, QS], FP32, tag="sT")
            for half in range(2):
                nc.tensor.matmul(
                    sT[:, 512 * half:512 * (half + 1)],
                    lhsT=k_sb[0:DH, h * N + 128 * kv:h * N + 128 * (kv + 1)],
                    rhs=q_sb[0:DH, h * N + q0 + 512 * half:
                             h * N + q0 + 512 * (half + 1)],
                    start=True, stop=True,
                )
            return sT

        def exp_(sT):
            pT = ppT.tile([128, QS], BF16_DT, tag="pT")
            nc.scalar.activation(
                out=pT, in_=sT,
                func=mybir.ActivationFunctionType.Exp,
                scale=float(SCALE),
            )
            return pT

        def acc_slot(accs, h, qt):
            if qt < 7:
                return accs[h], 65 * qt
            return accs[2], 65 * h

        def pv(accs, kv, h, pT):
            for qt in range(NQT):
                acc, off = acc_slot(accs, h, qt)
                # start=True clears has_written for the WHOLE psum bank, so
                # only the first slice written in each bank may use it; later
                # slices rely on that bank-wide clear (has_written=0 +
                # accumulate = direct write).
                first_in_bank = qt == 0 or (qt == 7 and h == 0)
                nc.tensor.matmul(
                    acc[:, off:off + 65],
                    lhsT=pT[:, 128 * qt:128 * (qt + 1)],
                    rhs=v_sb[:, kv, 65 * h:65 * (h + 1)],
                    start=(kv == 0 and first_in_bank),
                    stop=(kv == NKV - 1),
                    skip_group_check=True,
                )

        def norm_head(accs, qs, h, qts=range(NQT)):
            """Normalize head h's accumulators, transpose into oT_sb."""
            q0 = QS * qs
            for qt in qts:
                acc, off = acc_slot(accs, h, qt)
                rec = pnorm.tile([128, 1], FP32, tag="rec")
                nc.vector.reciprocal(rec, acc[:, off + 64:off + 65])
                o_sb = pnorm.tile([128, 64], BF16_DT, tag="o_sb")
                nc.vector.tensor_scalar_mul(
                    out=o_sb, in0=acc[:, off:off + 64], scalar1=rec)
                ps = pbig.tile([128, 128], BF16_DT, tag="big")
                nc.tensor.transpose(ps[0:64, :], o_sb, ident)
                nc.vector.tensor_copy(
                    out=oT_sb[0:64, h * N + q0 + 128 * qt:
                              h * N + q0 + 128 * (qt + 1)],
                    in_=ps[0:64, :],
                )

        def outproj_piece(ch, ct, pool=None):
            if pool is None:
                ps = pbig.tile([128, CH], FP32, tag="big", name="ps")
            else:
                ps = pool.tile([128, CH], FP32, tag="sT", name="ps")
            for h in range(2):
                nc.tensor.matmul(
                    ps,
                    lhsT=wo_sb[0:DH, h * C + 128 * ct:h * C + 128 * (ct + 1)],
                    rhs=oT_sb[0:DH, h * N + CH * ch:h * N + CH * (ch + 1)],
                    start=(h == 0), stop=(h == 1),
                )
            st = so.tile([128, CH], FP32, tag="st")
            nc.vector.tensor_scalar_add(
                out=st, in0=ps, scalar1=bo_sb[:, ct:ct + 1])
            nc.sync.dma_start(
                out=poutT[128 * ct:128 * (ct + 1), CH * ch:CH * (ch + 1)],
                in_=st,
            )

        # ---------- startup prefix ----------
        # (borrows the idle sT psum slots so chunks pipeline 3-wide)
        proj(k_sb, 2 * DH, 0, pool=psT)
        proj(q_sb, 0, 0, pool=psT)
        proj(q_sb, 0, 1)
        proj(vT_sb, 4 * DH, 0, pool=psT)
        for kv in range(4):
            vtr(kv, pool=psT if kv % 2 else None)

        # Filler tasks drip-fed into the attention loop's PE slack.
        # During qs0: remaining k/v/q projections + v transposes, ordered so
        # chunk j is fully emitted before iteration kv=4j needs it
        # (consumption is 2 tasks per kv iteration, twice the required rate).
        filler = deque()
        for j in range(1, NCH):
            filler.append(lambda j=j: proj(k_sb, 2 * DH, j))
            filler.append(lambda j=j: proj(vT_sb, 4 * DH, j))
            filler.append(lambda j=j: (vtr(4 * j), vtr(4 * j + 1)))
            filler.append(lambda j=j: (vtr(4 * j + 2), vtr(4 * j + 3)))
        for j in range(2, NCH):
            filler.append(lambda j=j: proj(q_sb, 0, j))

        def drain_filler(nmax):
            for _ in range(min(nmax, len(filler))):
                filler.popleft()()

        # ---------- attention (software-pipelined emission) ----------
        # Per iteration the ACT ops (exp h0, exp h1) are emitted first, and
        # the NEXT iteration's S^T matmuls are emitted right after each PV so
        # the scalar engine never waits on the PE stream.
        accs = [pacc.tile([128, 512], FP32, tag=t, name=t)
                for t in ("accA", "accB", "accC")]
        sT_next = [s_mm(0, 0, 0), s_mm(0, 0, 1)]
        for qs in range(NQS):
            last = qs == NQS - 1
            for kv in range(NKV):
                sT0, sT1 = sT_next
                pT0 = exp_(sT0)
                pT1 = exp_(sT1)
                sT_next = [None, None]
                pv(accs, kv, 0, pT0)
                if kv + 1 < NKV:
                    sT_next[0] = s_mm(qs, kv + 1, 0)
                elif not last:
                    sT_next[0] = s_mm(qs + 1, 0, 0)
                if kv == NKV - 1:
                    norm_head(accs, qs, 0)
                drain_filler(1)
                pv(accs, kv, 1, pT1)
                if kv + 1 < NKV:
                    sT_next[1] = s_mm(qs, kv + 1, 1)
                elif not last:
                    sT_next[1] = s_mm(qs + 1, 0, 1)
                if kv == NKV - 1 and not last:
                    norm_head(accs, qs, 1)

            if not last:
                accs = [pacc.tile([128, 512], FP32, tag=t, name=t)
                        for t in ("accA", "accB", "accC")]
                # output projection for this superblock's two 512-token
                # chunks, deferred as filler into the next superblock
                for ch in (2 * qs, 2 * qs + 1):
                    for ct in range(4):
                        filler.append(lambda ch=ch, ct=ct: outproj_piece(ch, ct))
            else:
                # tail: interleave the last norm with the output projection;
                # the sT slots are free (no more exps), so borrow them to
                # pipeline the pieces 3-wide
                norm_head(accs, qs, 1, range(0, 4))
                for ct in range(4):
                    outproj_piece(2 * qs, ct, pool=psT if ct % 2 else None)
                norm_head(accs, qs, 1, range(4, NQT))
                for ct in range(4):
                    outproj_piece(2 * qs + 1, ct, pool=psT if ct % 2 else None)
        assert not filler


_NC = None


def _build_nc():
    global _NC
    if _NC is None:
        nc = bacc.Bacc("TRN2", target_bir_lowering=False, debug=False,
                       num_devices=NCORES)
        with tile.TileContext(nc) as tc:
            _emit(tc)
        nc.finalize()
        _NC = nc
    return _NC


def _in_maps(x, w_qkv, b_qkv, w_out, b_out):
    x = np.asarray(x, dtype=np.float32)
    w_qkv = np.asarray(w_qkv, dtype=np.float32)
    b_qkv = np.asarray(b_qkv, dtype=np.float32)
    w_out = np.asarray(w_out, dtype=np.float32)
    b_out = np.asarray(b_out, dtype=np.float32)

    w4 = w_qkv.reshape(C, 3, H, DH)
    b4 = b_qkv.reshape(3, H, DH)
    xT_b = [np.ascontiguousarray(x[b].T).astype(BF16) for b in range(B)]

    maps = []
    for c in range(NCORES):
        b = c // 4
        h0, h1 = 2 * (c % 4), 2 * (c % 4) + 1
        wl = np.concatenate(
            [w4[:, 0, h0], w4[:, 0, h1], w4[:, 1, h0], w4[:, 1, h1],
             w4[:, 2, h0], w4[:, 2, h1]], axis=1).astype(BF16)
        bq = np.zeros((5, 128), np.float32)
        bq[0, :DH] = b4[0, h0]
        bq[1, DH:] = b4[0, h1]   # head1 bias lives on partitions 64-127
        bq[2, :DH] = b4[1, h0]
        bq[3, DH:] = b4[1, h1]
        bq[4] = np.concatenate([b4[2, h0], b4[2, h1]])
        wo = np.concatenate(
            [w_out[DH * h0:DH * (h0 + 1)], w_out[DH * h1:DH * (h1 + 1)]],
            axis=1).astype(BF16)
        bo = (b_out.reshape(4, 128) if c % 4 == 0
              else np.zeros((4, 128), np.float32))
        maps.append({
            "xT": xT_b[b],
            "wqkv": np.ascontiguousarray(wl),
            "bqkv": bq,
            "wout": np.ascontiguousarray(wo),
            "bout": np.ascontiguousarray(bo.astype(np.float32)),
        })
    return maps


def kernel(x, w_qkv, b_qkv, w_out, b_out, _trace=False, **_trace_kwargs):
    nc = _build_nc()
    maps = _in_maps(x, w_qkv, b_qkv, w_out, b_out)
    res = run_bass_kernel_spmd(nc, maps, core_ids=list(range(NCORES)),
                               trace=_trace, **_trace_kwargs)
    parts = [np.asarray(r["poutT"], dtype=np.float32) for r in res.results]
    out = np.empty((B, N, C), dtype=np.float32)
    for b in range(B):
        acc = parts[4 * b]
        for i in range(1, 4):
            acc = acc + parts[4 * b + i]
        out[b] = acc.T
    if _trace:
        return out, res
    return out

